# revision 4
# baseline (speedup 1.0000x reference)
"""Trainium2 Bass kernel for nn_BaseModel_2654289789315 (gnn_message_passing).

Strategy (validated numerically in fp64/fp32 on CPU):
  - The reference network's output depends only on the L=0 invariant channel.
    The L=1/L=2 uncoupled matrices are antisymmetric / traceless-symmetric, so
    the whole model reduces to per-(l,m) vectors f[atom, lm, 128] and traces:
        t_0 = (f0 @ W0) * f0 + f0
        t_l = s_l/sqrt(3) * sum_m (f_lm @ W_l) * f_lm   (s_1=-1, s_2=+1)
  - neigh features depend only on the neighbor's species (4 values) and
    R_l = rb @ W_rad, so the message-passing segment-sum only needs
        G[atom, lm, basis(8), species(4)]  (288 scalars per atom),
    computed on-device as a one-hot matmul scatter:
        G_block = sum_tiles V^T @ S   with V[pair,72]=sh x rb (outer product),
        S[pair,128] one-hot of (atom_in_block*4 + neighbor_species).
  - All 128-channel work happens in small dense per-atom matmuls.

Sharding: atoms (and their incident pairs, grouped by center) are sharded
across 8 cores; small weights are replicated; no collectives are needed
because each core owns all pairs of its atoms (neighbor data is materialized
per-shard on the host, i.e. the "halo exchange" happens at input-marshaling
time).
"""

import sys
if "/opt/trn_rl_repo" not in sys.path:
    sys.path.insert(0, "/opt/trn_rl_repo")

import math
import numpy as np

import concourse.bass as bass
import concourse.mybir as mybir
import concourse.tile as tile
from concourse import bacc, bass_utils

AF = mybir.ActivationFunctionType
ALU = mybir.AluOpType
DT = mybir.dt

# ---- problem constants (hardcoded per task spec) ----
N_ATOMS = 10000
N_PAIRS = 160000
N_TYPES = 4
N_CHANNELS = 32
N_MAX = 4
N_BASIS = 8
K = 128
L_MAX = 2
CUTOFF = 20.0
CUTOFF_WIDTH = 5.0
MP_SCALING = 0.1
K0_TOT = 384
NCORES = 8
NLOC = N_ATOMS // NCORES          # 1250 atoms per core
A_BLK = 32                         # atoms per scatter block
NBLK = math.ceil(NLOC / A_BLK)     # 40
NS = NBLK * A_BLK                  # 1280 output slots per core
P = 128
SQ3 = float(np.sqrt(3.0))
SIGMA = CUTOFF / N_BASIS           # 2.5
L_OF_LM = [0, 1, 1, 1, 2, 2, 2, 2, 2]

# dtype config: stage-wise float32r (PE fast path, ~1e-4 relative rounding)
F32R_SCATTER = False
F32R_F = False
F32R_CG = False
F32R_HEAD = False

_BUILD_CACHE = {}


def _build(TPB):
    """Build + compile the single-core Bass program (SPMD across 8 cores)."""
    T = NBLK * TPB                # total pair tiles
    BPC = 8                       # blocks per pair-stage chunk
    NCH = NBLK // BPC             # 5 chunks
    TC = BPC * TPB                # tiles per chunk

    nc = bacc.Bacc("TRN2", target_bir_lowering=False, debug=False,
                   num_devices=NCORES)

    def din(name, shape, dt=DT.float32):
        return nc.dram_tensor(name, shape, dt, kind="ExternalInput")

    posnb_d = din("posnb", [P, T, 3])
    posct_d = din("posct", [P, T, 3])
    colf_d = din("colf", [P, T])
    specr_d = din("specr", [N_TYPES, NS])
    iota_d = din("iota", [P, P])
    mu_d = din("mu", [P, N_BASIS])
    mcol_d = din("mcol", [72, 36 * K])
    wcg_d = din("wcg", [K, 3 * K])
    eexp_d = din("eexp", [N_TYPES, K0_TOT])
    whead_d = din("whead", [3, K, K0_TOT])
    bhead_d = din("bhead", [K, 3])
    wout_d = din("wout", [K, 3])
    bout_d = din("bout", [1, 1])
    svals_d = din("svals", [N_TYPES, 1])
    out_d = nc.dram_tensor("out", [1, NS], DT.float32, kind="ExternalOutput")

    f32 = DT.float32
    r_sc = DT.float32r if F32R_SCATTER else f32
    r_f = DT.float32r if F32R_F else f32
    r_cg = DT.float32r if F32R_CG else f32
    r_hd = DT.float32r if F32R_HEAD else f32

    with tile.TileContext(nc) as tc:
        with tc.tile_pool(name="const", bufs=1) as cp, \
             tc.tile_pool(name="gpool", bufs=1) as gp, \
             tc.tile_pool(name="psum", bufs=2, space="PSUM") as pp:

            # ---- constants into SBUF ----
            iota_sb = cp.tile([P, P], f32)
            nc.sync.dma_start(iota_sb[:], iota_d.ap())
            mu_sb = cp.tile([P, N_BASIS], f32)
            nc.sync.dma_start(mu_sb[:], mu_d.ap())
            mcol_sb = cp.tile([72, 36 * K], r_f)
            if F32R_F:
                mcol_f32 = cp.tile([72, 36 * K], f32)
                nc.sync.dma_start(mcol_f32[:], mcol_d.ap())
                nc.vector.tensor_copy(mcol_sb[:], mcol_f32[:])
            else:
                nc.sync.dma_start(mcol_sb[:], mcol_d.ap())
            wcg_sb = cp.tile([K, 3 * K], r_cg)
            if F32R_CG:
                wcg_f32 = cp.tile([K, 3 * K], f32)
                nc.sync.dma_start(wcg_f32[:], wcg_d.ap())
                nc.vector.tensor_copy(wcg_sb[:], wcg_f32[:])
            else:
                nc.sync.dma_start(wcg_sb[:], wcg_d.ap())
            eexp_sb = cp.tile([N_TYPES, K0_TOT], f32)
            nc.sync.dma_start(eexp_sb[:], eexp_d.ap())
            whead_sb = [cp.tile([K, K0_TOT], r_hd, name=f"whead{i}", tag=f"whead{i}") for i in range(3)]
            for i in range(3):
                if F32R_HEAD:
                    wtmp = cp.tile([K, K0_TOT], f32, tag=f"wheadf{i}")
                    nc.sync.dma_start(wtmp[:], whead_d.ap()[i])
                    nc.vector.tensor_copy(whead_sb[i][:], wtmp[:])
                else:
                    nc.sync.dma_start(whead_sb[i][:], whead_d.ap()[i])
            bhead_sb = cp.tile([K, 3], f32)
            nc.sync.dma_start(bhead_sb[:], bhead_d.ap())
            wout_sb = cp.tile([K, 3], f32)
            nc.sync.dma_start(wout_sb[:], wout_d.ap())
            bout_sb = cp.tile([1, 1], f32)
            nc.sync.dma_start(bout_sb[:], bout_d.ap())
            specr_sb = cp.tile([N_TYPES, NS], f32)
            nc.sync.dma_start(specr_sb[:], specr_d.ap())
            svals_sb = cp.tile([N_TYPES, 1], f32)
            nc.sync.dma_start(svals_sb[:], svals_d.ap())

            def bias_tile(val, tag):
                bt = cp.tile([P, 1], f32, tag=tag)
                nc.vector.memset(bt[:], val)
                return bt

            b_eps = bias_tile(1e-12, "b_eps")
            b_half_pi = bias_tile(float(np.pi / 2), "b_hpi")
            b_zero = bias_tile(0.0, "b_zero")

            # ---- G accumulator in SBUF ----
            g_sb = gp.tile([72, NBLK * P], r_f)

            # ================= pair stage =================
            with tc.tile_pool(name="pair", bufs=2) as wp:
                for ch in range(NCH):
                    t0 = ch * TC
                    pnb = wp.tile([P, TC, 3], f32)
                    nc.sync.dma_start(pnb[:], posnb_d.ap()[:, t0:t0 + TC, :])
                    pct = wp.tile([P, TC, 3], f32)
                    nc.sync.dma_start(pct[:], posct_d.ap()[:, t0:t0 + TC, :])
                    col = wp.tile([P, TC], f32)
                    nc.sync.dma_start(col[:], colf_d.ap()[:, t0:t0 + TC])

                    rv = wp.tile([P, TC, 3], f32)
                    nc.vector.tensor_tensor(out=rv[:], in0=pnb[:], in1=pct[:],
                                            op=ALU.subtract)
                    rr = wp.tile([P, TC], f32)
                    nc.vector.tensor_tensor(out=rr[:], in0=rv[:, :, 0],
                                            in1=rv[:, :, 0], op=ALU.mult)
                    tmp2 = wp.tile([P, TC], f32)
                    nc.vector.tensor_tensor(out=tmp2[:], in0=rv[:, :, 1],
                                            in1=rv[:, :, 1], op=ALU.mult)
                    nc.vector.tensor_tensor(out=rr[:], in0=rr[:], in1=tmp2[:],
                                            op=ALU.add)
                    nc.vector.tensor_tensor(out=tmp2[:], in0=rv[:, :, 2],
                                            in1=rv[:, :, 2], op=ALU.mult)
                    nc.vector.tensor_tensor(out=rr[:], in0=rr[:], in1=tmp2[:],
                                            op=ALU.add)
                    lnrr = wp.tile([P, TC], f32)
                    nc.scalar.activation(lnrr[:], rr[:], AF.Ln,
                                         bias=b_eps[:], scale=1.0)
                    dd = wp.tile([P, TC], f32)
                    nc.scalar.activation(dd[:], lnrr[:], AF.Exp,
                                         bias=b_zero[:], scale=0.5)
                    invd = wp.tile([P, TC], f32)
                    nc.scalar.activation(invd[:], lnrr[:], AF.Exp,
                                         bias=b_zero[:], scale=-0.5)
                    uv = wp.tile([P, TC, 3], f32)
                    nc.vector.tensor_tensor(
                        out=uv[:], in0=rv[:],
                        in1=invd[:].unsqueeze(2).to_broadcast([P, TC, 3]),
                        op=ALU.mult)

                    # sh planes (lm=1..8): y z x, s3*xy, s3*yz, .5(3zz-1), s3*xz, .5*s3(xx-yy)
                    sh = wp.tile([P, 8, TC], f32)
                    ux, uy, uz = uv[:, :, 0], uv[:, :, 1], uv[:, :, 2]
                    nc.vector.tensor_copy(sh[:, 0, :], uy)
                    nc.vector.tensor_copy(sh[:, 1, :], uz)
                    nc.vector.tensor_copy(sh[:, 2, :], ux)
                    nc.vector.scalar_tensor_tensor(out=sh[:, 3, :], in0=ux,
                                                   scalar=SQ3, in1=uy,
                                                   op0=ALU.mult, op1=ALU.mult)
                    nc.vector.scalar_tensor_tensor(out=sh[:, 4, :], in0=uy,
                                                   scalar=SQ3, in1=uz,
                                                   op0=ALU.mult, op1=ALU.mult)
                    zz3 = wp.tile([P, TC], f32)
                    nc.vector.scalar_tensor_tensor(out=zz3[:], in0=uz,
                                                   scalar=3.0, in1=uz,
                                                   op0=ALU.mult, op1=ALU.mult)
                    nc.vector.tensor_scalar(out=sh[:, 5, :], in0=zz3[:],
                                            scalar1=0.5, scalar2=-0.5,
                                            op0=ALU.mult, op1=ALU.add)
                    nc.vector.scalar_tensor_tensor(out=sh[:, 6, :], in0=ux,
                                                   scalar=SQ3, in1=uz,
                                                   op0=ALU.mult, op1=ALU.mult)
                    xx = wp.tile([P, TC], f32)
                    nc.vector.scalar_tensor_tensor(out=xx[:], in0=ux,
                                                   scalar=0.5 * SQ3, in1=ux,
                                                   op0=ALU.mult, op1=ALU.mult)
                    yy = wp.tile([P, TC], f32)
                    nc.vector.scalar_tensor_tensor(out=yy[:], in0=uy,
                                                   scalar=0.5 * SQ3, in1=uy,
                                                   op0=ALU.mult, op1=ALU.mult)
                    nc.vector.tensor_tensor(out=sh[:, 7, :], in0=xx[:],
                                            in1=yy[:], op=ALU.subtract)

                    # radial basis rb[b, t] = exp(-((d-mu_b)/sigma)^2) * fc(d)
                    ev = wp.tile([P, N_BASIS, TC], f32)
                    nc.vector.tensor_tensor(
                        out=ev[:],
                        in0=dd[:].unsqueeze(1).to_broadcast([P, N_BASIS, TC]),
                        in1=mu_sb[:].unsqueeze(2).to_broadcast([P, N_BASIS, TC]),
                        op=ALU.subtract)
                    e2 = wp.tile([P, N_BASIS, TC], f32)
                    nc.vector.tensor_tensor(out=e2[:], in0=ev[:], in1=ev[:],
                                            op=ALU.mult)
                    gauss = wp.tile([P, N_BASIS, TC], f32)
                    nc.scalar.activation(gauss[:], e2[:], AF.Exp,
                                         bias=b_zero[:],
                                         scale=-1.0 / (SIGMA * SIGMA))
                    tcv = wp.tile([P, TC], f32)
                    nc.vector.tensor_scalar(out=tcv[:], in0=dd[:],
                                            scalar1=CUTOFF - CUTOFF_WIDTH,
                                            scalar2=1.0 / CUTOFF_WIDTH,
                                            op0=ALU.subtract, op1=ALU.mult)
                    nc.vector.tensor_scalar(out=tcv[:], in0=tcv[:],
                                            scalar1=0.0, scalar2=1.0,
                                            op0=ALU.max, op1=ALU.min)
                    cosv = wp.tile([P, TC], f32)
                    nc.scalar.activation(cosv[:], tcv[:], AF.Sin,
                                         bias=b_half_pi[:],
                                         scale=-float(np.pi))
                    fc = wp.tile([P, TC], f32)
                    nc.vector.tensor_scalar(out=fc[:], in0=cosv[:],
                                            scalar1=0.5, scalar2=0.5,
                                            op0=ALU.mult, op1=ALU.add)
                    rb = wp.tile([P, N_BASIS, TC], f32)
                    nc.vector.tensor_tensor(
                        out=rb[:], in0=gauss[:],
                        in1=fc[:].unsqueeze(1).to_broadcast([P, N_BASIS, TC]),
                        op=ALU.mult)

                    # V[p, t, 72]: cols lm*8+b; lm=0 block is rb itself
                    vt = wp.tile([P, TC, 72], r_sc)
                    nc.vector.tensor_copy(
                        vt[:, :, 0:8],
                        rb[:].rearrange("p b t -> p t b"))
                    nc.vector.tensor_tensor(
                        out=vt[:, :, 8:72].rearrange("p t (lm b) -> p t lm b",
                                                     lm=8, b=8),
                        in0=sh[:].rearrange("p lm t -> p t lm")
                                 .unsqueeze(3).to_broadcast([P, TC, 8, 8]),
                        in1=rb[:].rearrange("p b t -> p t b")
                                 .unsqueeze(2).to_broadcast([P, TC, 8, 8]),
                        op=ALU.mult)

                    # one-hot S[p, t, 128] of col index
                    st = wp.tile([P, TC, P], r_sc)
                    nc.vector.tensor_tensor(
                        out=st[:],
                        in0=col[:].unsqueeze(2).to_broadcast([P, TC, P]),
                        in1=iota_sb[:].unsqueeze(1).to_broadcast([P, TC, P]),
                        op=ALU.is_equal)

                    # scatter matmuls: G_blk [72, 128] += V_t^T @ S_t
                    for bl in range(BPC):
                        b = ch * BPC + bl
                        psg = pp.tile([72, P], f32, space="PSUM", tag="psG")
                        for j in range(TPB):
                            tt = bl * TPB + j
                            nc.tensor.matmul(out=psg[:],
                                             lhsT=vt[:, tt, :],
                                             rhs=st[:, tt, :],
                                             start=(j == 0),
                                             stop=(j == TPB - 1))
                        nc.scalar.copy(g_sb[:, b * P:(b + 1) * P], psg[:])

            # ================= atom stage =================
            with tc.tile_pool(name="atom", bufs=1) as ap:
                # f matmuls: fT[k, lm, slot]
                g4 = g_sb[:].rearrange("p (blk a s) -> p blk a s",
                                       a=A_BLK, s=N_TYPES)
                csizes = []
                c0 = 0
                while c0 < NBLK:
                    cb = min(16, NBLK - c0)
                    csizes.append((c0, cb))
                    c0 += cb
                ft = ap.tile([K, 9, NS], r_cg)
                for lm in range(9):
                    for (cb0, cbn) in csizes:
                        psf = pp.tile([K, 512], f32, space="PSUM", tag="ps512")
                        n = cbn * A_BLK
                        for s in range(N_TYPES):
                            nc.tensor.matmul(
                                out=psf[:, 0:n],
                                lhsT=mcol_sb[:, (lm * 4 + s) * K:(lm * 4 + s + 1) * K],
                                rhs=g4[:, cb0:cb0 + cbn, :, s],
                                start=(s == 0), stop=(s == N_TYPES - 1))
                        nc.scalar.copy(ft[:, lm, cb0 * A_BLK:cb0 * A_BLK + n],
                                       psf[:, 0:n])

                # CG stage: tl[k, l, slot]
                tl = ap.tile([K, 3, NS], f32)
                tmp = ap.tile([K, 512], f32)
                for l in range(3):
                    lms = [i for i in range(9) if L_OF_LM[i] == l]
                    for (cb0, cbn) in csizes:
                        n = cbn * A_BLK
                        sl = slice(cb0 * A_BLK, cb0 * A_BLK + n)
                        for mi, lm in enumerate(lms):
                            psc = pp.tile([K, 512], f32, space="PSUM",
                                          tag="ps512")
                            nc.tensor.matmul(
                                out=psc[:, 0:n],
                                lhsT=wcg_sb[:, l * K:(l + 1) * K],
                                rhs=ft[:, lm, sl],
                                start=True, stop=True)
                            if mi == 0:
                                nc.vector.tensor_tensor(
                                    out=tl[:, l, sl], in0=psc[:, 0:n],
                                    in1=ft[:, lm, sl], op=ALU.mult)
                            else:
                                nc.vector.tensor_tensor(
                                    out=tmp[:, 0:n], in0=psc[:, 0:n],
                                    in1=ft[:, lm, sl], op=ALU.mult)
                                nc.vector.tensor_tensor(
                                    out=tl[:, l, sl], in0=tl[:, l, sl],
                                    in1=tmp[:, 0:n], op=ALU.add)
                        if l == 0:
                            nc.vector.tensor_tensor(
                                out=tl[:, 0, sl], in0=tl[:, 0, sl],
                                in1=ft[:, 0, sl], op=ALU.add)

                # species one-hot of centers and embedding factor
                oct_sb = ap.tile([N_TYPES, NS], f32)
                nc.vector.tensor_tensor(
                    out=oct_sb[:], in0=specr_sb[:],
                    in1=svals_sb[:].to_broadcast([N_TYPES, NS]),
                    op=ALU.is_equal)
                x0e = ap.tile([K, 3, NS], r_hd)
                for l in range(3):
                    for (cb0, cbn) in csizes:
                        n = cbn * A_BLK
                        sl = slice(cb0 * A_BLK, cb0 * A_BLK + n)
                        pse = pp.tile([K, 512], f32, space="PSUM", tag="ps512")
                        nc.tensor.matmul(out=pse[:, 0:n],
                                         lhsT=eexp_sb[:, l * K:(l + 1) * K],
                                         rhs=oct_sb[:, sl],
                                         start=True, stop=True)
                        nc.vector.tensor_tensor(out=x0e[:, l, sl],
                                                in0=pse[:, 0:n],
                                                in1=tl[:, l, sl], op=ALU.mult)

                # head: hT[j, slot] = silu(sum_R Whead[R, j] x0e[R, slot] + b)
                ht = ap.tile([K, 3, NS], r_hd)
                for jc in range(3):
                    for (cb0, cbn) in csizes:
                        n = cbn * A_BLK
                        sl = slice(cb0 * A_BLK, cb0 * A_BLK + n)
                        psh = pp.tile([K, 512], f32, space="PSUM", tag="ps512")
                        for rc in range(3):
                            nc.tensor.matmul(
                                out=psh[:, 0:n],
                                lhsT=whead_sb[rc][:, jc * K:(jc + 1) * K],
                                rhs=x0e[:, rc, sl],
                                start=(rc == 0), stop=(rc == 2))
                        nc.scalar.activation(ht[:, jc, sl], psh[:, 0:n],
                                             AF.Silu,
                                             bias=bhead_sb[:, jc:jc + 1],
                                             scale=1.0)

                # out row
                outsb = ap.tile([1, NS], f32)
                for (cb0, cbn) in csizes:
                    n = cbn * A_BLK
                    sl = slice(cb0 * A_BLK, cb0 * A_BLK + n)
                    pso = pp.tile([1, 512], f32, space="PSUM", tag="psO")
                    for rc in range(3):
                        nc.tensor.matmul(out=pso[:, 0:n],
                                         lhsT=wout_sb[:, rc:rc + 1],
                                         rhs=ht[:, rc, sl],
                                         start=(rc == 0), stop=(rc == 2))
                    nc.scalar.activation(outsb[:, sl], pso[:, 0:n],
                                         AF.Identity,
                                         bias=bout_sb[:], scale=1.0)
                nc.sync.dma_start(out_d.ap(), outsb[:])

    nc.compile()
    return nc, T


def _prep_inputs(inputs, TPB):
    """Host-side sharding: sort pairs by center, bucket into per-core,
    per-block tile slots, and materialize per-pair endpoint positions."""
    T = NBLK * TPB
    pos = np.ascontiguousarray(np.asarray(inputs["positions"], np.float32))
    spec = np.asarray(inputs["species"]).astype(np.int64)
    pairs = np.asarray(inputs["pairs"]).astype(np.int64)
    ctr, nbr = pairs[:, 0], pairs[:, 1]
    order = np.argsort(ctr, kind="stable")
    ctr = ctr[order]
    nbr = nbr[order]
    spec_nb = spec[nbr].astype(np.float32)

    core = ctr // NLOC
    loc = ctr - core * NLOC
    blk = loc // A_BLK
    arel = loc - blk * A_BLK

    # rank within (core, block)
    key = core * NBLK + blk
    # pairs sorted by ctr -> key is non-decreasing
    counts = np.bincount(key, minlength=NCORES * NBLK)
    starts = np.concatenate([[0], np.cumsum(counts)[:-1]])
    rank = np.arange(len(ctr)) - starts[key]

    slot = blk * (TPB * P) + rank          # slot within core's pair arrays
    tt = slot // P
    qq = slot - tt * P

    in_maps = []
    # constant tables (shared across cores)
    iota_np = np.broadcast_to(np.arange(P, dtype=np.float32), (P, P)).copy()
    mu_np = np.broadcast_to(
        np.linspace(0.0, CUTOFF, N_BASIS, dtype=np.float32), (P, N_BASIS)).copy()

    emb = np.asarray(inputs["embeddings"], np.float32)
    h0t = np.repeat(emb, N_MAX, axis=1)                    # [4, 128]
    W_rad = np.asarray(inputs["W_rad"], np.float32)
    mcol = np.zeros((72, 36 * K), np.float32)
    for lm in range(9):
        l = L_OF_LM[lm]
        for s in range(N_TYPES):
            blkc = (lm * 4 + s) * K
            for b in range(N_BASIS):
                mcol[lm * 8 + b, blkc:blkc + K] = \
                    MP_SCALING * W_rad[l, b, :] * h0t[s, :]
    wcg = np.concatenate([
        np.asarray(inputs["W_cg0"], np.float32),
        np.asarray(inputs["W_cg1"], np.float32) * np.float32(-1.0 / SQ3),
        np.asarray(inputs["W_cg2"], np.float32) * np.float32(1.0 / SQ3),
    ], axis=1)                                             # [128, 384]
    eexp = np.repeat(emb, K0_TOT // N_CHANNELS, axis=1)    # [4, 384]
    W_head = np.asarray(inputs["W_head"], np.float32)      # [384, 384]
    whead = np.stack([W_head[i * K:(i + 1) * K, :] for i in range(3)])
    b_head = np.asarray(inputs["b_head"], np.float32)
    bhead = b_head.reshape(3, K).T.copy()                  # [128, 3]
    W_out = np.asarray(inputs["W_out"], np.float32)        # [384, 1]
    wout = W_out[:, 0].reshape(3, K).T.copy()              # [128, 3]
    bout = np.asarray(inputs["b_out"], np.float32).reshape(1, 1)

    for c in range(NCORES):
        m = core == c
        posnb = np.zeros((P, T, 3), np.float32)
        posct = np.zeros((P, T, 3), np.float32)
        colf = np.full((P, T), -1.0, np.float32)
        posnb[qq[m], tt[m]] = pos[nbr[m]]
        posct[qq[m], tt[m]] = pos[ctr[m]]
        colf[qq[m], tt[m]] = arel[m] * N_TYPES + spec_nb[m]
        slots = np.arange(NS)
        atom = c * NLOC + np.minimum(slots, NLOC - 1)
        specr = np.broadcast_to(spec[atom].astype(np.float32), (N_TYPES, NS)).copy()
        in_maps.append(dict(
            posnb=posnb, posct=posct, colf=colf, specr=specr,
            iota=iota_np, mu=mu_np, mcol=mcol, wcg=wcg, eexp=eexp,
            whead=whead, bhead=bhead, wout=wout, bout=bout,
            svals=np.arange(N_TYPES, dtype=np.float32).reshape(N_TYPES, 1),
        ))
    return in_maps


def _required_tpb(inputs):
    pairs = np.asarray(inputs["pairs"]).astype(np.int64)
    ctr = pairs[:, 0]
    key = (ctr // NLOC) * NBLK + (ctr % NLOC) // A_BLK
    counts = np.bincount(key, minlength=NCORES * NBLK)
    return max(5, int(math.ceil(counts.max() / P)))


def _install_ntff_hook():
    """Provide the antenv.axon_hooks registry this image lacks, backed by
    direct ctypes calls into libaxon_pjrt.so (same mechanism trn_boot uses)."""
    import types
    if "antenv.axon_hooks" in sys.modules:
        return
    try:
        import antenv
        from trn_agent_boot.trn_boot import _ntff_profile_via_ctypes
        hook = _ntff_profile_via_ctypes("/opt/axon/libaxon_pjrt.so")
        mod = types.ModuleType("antenv.axon_hooks")
        _h = {"hook": hook}
        mod.get_axon_ntff_profile_hook = lambda: _h["hook"]
        mod.set_axon_ntff_profile_hook = lambda h: _h.__setitem__("hook", h)
        sys.modules["antenv.axon_hooks"] = mod
        antenv.axon_hooks = mod
        bass_utils.upload_artifacts = lambda d: f"file://{d}"
    except Exception as e:
        print("ntff hook install failed:", repr(e))


def run_cores(inputs, trace=False):
    if trace:
        _install_ntff_hook()
    TPB = _required_tpb(inputs)
    if TPB not in _BUILD_CACHE:
        _BUILD_CACHE[TPB] = _build(TPB)
    nc, T = _BUILD_CACHE[TPB]
    in_maps = _prep_inputs(inputs, TPB)
    res = bass_utils.run_bass_kernel_spmd(
        nc, in_maps, core_ids=list(range(NCORES)), trace=trace)
    outs = [res.results[c]["out"][0, :NLOC] for c in range(NCORES)]
    full = np.concatenate(outs).reshape(N_ATOMS, 1).astype(np.float32)
    return full, res


def kernel(**inputs):
    full, _ = run_cores(inputs, trace=False)
    return full


# revision 6
# speedup vs baseline: 1.6339x; 1.6339x over previous
"""Trainium2 Bass kernel for nn_BaseModel_2654289789315 (gnn_message_passing).

Strategy (validated numerically in fp64/fp32 on CPU):
  - The reference network's output depends only on the L=0 invariant channel.
    The L=1/L=2 uncoupled matrices are antisymmetric / traceless-symmetric, so
    the whole model reduces to per-(l,m) vectors f[atom, lm, 128] and traces:
        t_0 = (f0 @ W0) * f0 + f0
        t_l = s_l/sqrt(3) * sum_m (f_lm @ W_l) * f_lm   (s_1=-1, s_2=+1)
  - neigh features depend only on the neighbor's species (4 values) and
    R_l = rb @ W_rad, so the message-passing segment-sum only needs
        G[atom, lm, basis(8), species(4)]  (288 scalars per atom),
    computed on-device as a one-hot matmul scatter:
        G_block = sum_tiles V^T @ S   with V[pair,72]=sh x rb (outer product),
        S[pair,128] one-hot of (atom_in_block*4 + neighbor_species).
  - All 128-channel work happens in small dense per-atom matmuls.

Sharding: atoms (and their incident pairs, grouped by center) are sharded
across 8 cores; small weights are replicated; no collectives are needed
because each core owns all pairs of its atoms (neighbor data is materialized
per-shard on the host, i.e. the "halo exchange" happens at input-marshaling
time).
"""

import sys
if "/opt/trn_rl_repo" not in sys.path:
    sys.path.insert(0, "/opt/trn_rl_repo")

import math
import numpy as np

import concourse.bass as bass
import concourse.mybir as mybir
import concourse.tile as tile
from concourse import bacc, bass_utils

AF = mybir.ActivationFunctionType
ALU = mybir.AluOpType
DT = mybir.dt

# ---- problem constants (hardcoded per task spec) ----
N_ATOMS = 10000
N_PAIRS = 160000
N_TYPES = 4
N_CHANNELS = 32
N_MAX = 4
N_BASIS = 8
K = 128
L_MAX = 2
CUTOFF = 20.0
CUTOFF_WIDTH = 5.0
MP_SCALING = 0.1
K0_TOT = 384
NCORES = 8
NLOC = N_ATOMS // NCORES          # 1250 atoms per core
A_BLK = 32                         # atoms per scatter block
NBLK = math.ceil(NLOC / A_BLK)     # 40
NS = NBLK * A_BLK                  # 1280 output slots per core
P = 128
SQ3 = float(np.sqrt(3.0))
SIGMA = CUTOFF / N_BASIS           # 2.5
L_OF_LM = [0, 1, 1, 1, 2, 2, 2, 2, 2]

# dtype config: stage-wise float32r (PE fast path, ~1e-4 relative rounding)
F32R_SCATTER = False
F32R_F = True
F32R_CG = True
F32R_HEAD = True

_BUILD_CACHE = {}


def _build(TPB):
    """Build + compile the single-core Bass program (SPMD across 8 cores)."""
    T = NBLK * TPB                # total pair tiles
    BPC = 8                       # blocks per pair-stage chunk
    NCH = NBLK // BPC             # 5 chunks
    TC = BPC * TPB                # tiles per chunk

    nc = bacc.Bacc("TRN2", target_bir_lowering=False, debug=False,
                   num_devices=NCORES)

    def din(name, shape, dt=DT.float32):
        return nc.dram_tensor(name, shape, dt, kind="ExternalInput")

    posnb_d = din("posnb", [P, T, 3])
    posct_d = din("posct", [P, T, 3])
    colf_d = din("colf", [P, T])
    specr_d = din("specr", [N_TYPES, NS])
    iota_d = din("iota", [P, P])
    mu_d = din("mu", [P, N_BASIS])
    mcol_d = din("mcol", [72, 36 * K])
    wcg_d = din("wcg", [K, 3 * K])
    eexp_d = din("eexp", [N_TYPES, K0_TOT])
    whead_d = din("whead", [3, K, K0_TOT])
    bhead_d = din("bhead", [K, 3])
    wout_d = din("wout", [K, 3])
    bout_d = din("bout", [1, 1])
    svals_d = din("svals", [N_TYPES, 1])
    out_d = nc.dram_tensor("out", [1, NS], DT.float32, kind="ExternalOutput")

    f32 = DT.float32
    r_sc = DT.float32r if F32R_SCATTER else f32
    r_f = DT.float32r if F32R_F else f32
    r_cg = DT.float32r if F32R_CG else f32
    r_hd = DT.float32r if F32R_HEAD else f32

    with tile.TileContext(nc) as tc:
        with tc.tile_pool(name="const", bufs=1) as cp, \
             tc.tile_pool(name="gpool", bufs=1) as gp, \
             tc.tile_pool(name="psum", bufs=2, space="PSUM") as pp:

            # ---- constants into SBUF ----
            iota_sb = cp.tile([P, P], f32)
            nc.sync.dma_start(iota_sb[:], iota_d.ap())
            mu_sb = cp.tile([P, N_BASIS], f32)
            nc.sync.dma_start(mu_sb[:], mu_d.ap())
            mcol_sb = cp.tile([72, 36 * K], r_f)
            if F32R_F:
                mcol_f32 = cp.tile([72, 36 * K], f32)
                nc.sync.dma_start(mcol_f32[:], mcol_d.ap())
                nc.vector.tensor_copy(mcol_sb[:], mcol_f32[:])
            else:
                nc.sync.dma_start(mcol_sb[:], mcol_d.ap())
            wcg_sb = cp.tile([K, 3 * K], r_cg)
            if F32R_CG:
                wcg_f32 = cp.tile([K, 3 * K], f32)
                nc.sync.dma_start(wcg_f32[:], wcg_d.ap())
                nc.vector.tensor_copy(wcg_sb[:], wcg_f32[:])
            else:
                nc.sync.dma_start(wcg_sb[:], wcg_d.ap())
            eexp_sb = cp.tile([N_TYPES, K0_TOT], f32)
            nc.sync.dma_start(eexp_sb[:], eexp_d.ap())
            whead_sb = [cp.tile([K, K0_TOT], r_hd, name=f"whead{i}", tag=f"whead{i}") for i in range(3)]
            for i in range(3):
                if F32R_HEAD:
                    wtmp = cp.tile([K, K0_TOT], f32, tag=f"wheadf{i}")
                    nc.sync.dma_start(wtmp[:], whead_d.ap()[i])
                    nc.vector.tensor_copy(whead_sb[i][:], wtmp[:])
                else:
                    nc.sync.dma_start(whead_sb[i][:], whead_d.ap()[i])
            bhead_sb = cp.tile([K, 3], f32)
            nc.sync.dma_start(bhead_sb[:], bhead_d.ap())
            wout_sb = cp.tile([K, 3], r_hd)
            if F32R_HEAD:
                wout_f32 = cp.tile([K, 3], f32)
                nc.sync.dma_start(wout_f32[:], wout_d.ap())
                nc.vector.tensor_copy(wout_sb[:], wout_f32[:])
            else:
                nc.sync.dma_start(wout_sb[:], wout_d.ap())
            bout_sb = cp.tile([1, 1], f32)
            nc.sync.dma_start(bout_sb[:], bout_d.ap())
            specr_sb = cp.tile([N_TYPES, NS], f32)
            nc.sync.dma_start(specr_sb[:], specr_d.ap())
            svals_sb = cp.tile([N_TYPES, 1], f32)
            nc.sync.dma_start(svals_sb[:], svals_d.ap())

            def bias_tile(val, tag):
                bt = cp.tile([P, 1], f32, tag=tag)
                nc.vector.memset(bt[:], val)
                return bt

            b_eps = bias_tile(1e-12, "b_eps")
            b_half_pi = bias_tile(float(np.pi / 2), "b_hpi")
            b_zero = bias_tile(0.0, "b_zero")

            # ---- G accumulator in SBUF ----
            g_sb = gp.tile([72, NBLK * P], r_f)

            # ================= pair stage =================
            # Phase A/B: whole-T geometry + activations batched by table set
            # (Ln/Exp together, then Sin) to avoid ACT table thrash.
            with tc.tile_pool(name="geom", bufs=1) as gm:
                pnb = gm.tile([P, T, 3], f32)
                nc.sync.dma_start(pnb[:], posnb_d.ap())
                pct = gm.tile([P, T, 3], f32)
                nc.sync.dma_start(pct[:], posct_d.ap())
                colT = gm.tile([P, T], f32)
                nc.sync.dma_start(colT[:], colf_d.ap())

                rv = gm.tile([P, T, 3], f32)
                nc.vector.tensor_tensor(out=rv[:], in0=pnb[:], in1=pct[:],
                                        op=ALU.subtract)
                rr = gm.tile([P, T], f32)
                nc.vector.tensor_tensor(out=rr[:], in0=rv[:, :, 0],
                                        in1=rv[:, :, 0], op=ALU.mult)
                tmp2 = gm.tile([P, T], f32)
                nc.vector.tensor_tensor(out=tmp2[:], in0=rv[:, :, 1],
                                        in1=rv[:, :, 1], op=ALU.mult)
                nc.vector.tensor_tensor(out=rr[:], in0=rr[:], in1=tmp2[:],
                                        op=ALU.add)
                nc.vector.tensor_tensor(out=tmp2[:], in0=rv[:, :, 2],
                                        in1=rv[:, :, 2], op=ALU.mult)
                nc.vector.tensor_tensor(out=rr[:], in0=rr[:], in1=tmp2[:],
                                        op=ALU.add)
                lnrr = gm.tile([P, T], f32)
                nc.scalar.activation(lnrr[:], rr[:], AF.Ln,
                                     bias=b_eps[:], scale=1.0)
                dd = gm.tile([P, T], f32)
                nc.scalar.activation(dd[:], lnrr[:], AF.Exp,
                                     bias=b_zero[:], scale=0.5)
                invd = gm.tile([P, T], f32)
                nc.scalar.activation(invd[:], lnrr[:], AF.Exp,
                                     bias=b_zero[:], scale=-0.5)
                uv = gm.tile([P, T, 3], f32)
                nc.vector.tensor_tensor(
                    out=uv[:], in0=rv[:],
                    in1=invd[:].unsqueeze(2).to_broadcast([P, T, 3]),
                    op=ALU.mult)

                sh = gm.tile([P, 8, T], f32)
                ux, uy, uz = uv[:, :, 0], uv[:, :, 1], uv[:, :, 2]
                nc.vector.tensor_copy(sh[:, 0, :], uy)
                nc.vector.tensor_copy(sh[:, 1, :], uz)
                nc.vector.tensor_copy(sh[:, 2, :], ux)
                nc.vector.scalar_tensor_tensor(out=sh[:, 3, :], in0=ux,
                                               scalar=SQ3, in1=uy,
                                               op0=ALU.mult, op1=ALU.mult)
                nc.vector.scalar_tensor_tensor(out=sh[:, 4, :], in0=uy,
                                               scalar=SQ3, in1=uz,
                                               op0=ALU.mult, op1=ALU.mult)
                zz3 = gm.tile([P, T], f32)
                nc.vector.scalar_tensor_tensor(out=zz3[:], in0=uz,
                                               scalar=3.0, in1=uz,
                                               op0=ALU.mult, op1=ALU.mult)
                nc.vector.tensor_scalar(out=sh[:, 5, :], in0=zz3[:],
                                        scalar1=0.5, scalar2=-0.5,
                                        op0=ALU.mult, op1=ALU.add)
                nc.vector.scalar_tensor_tensor(out=sh[:, 6, :], in0=ux,
                                               scalar=SQ3, in1=uz,
                                               op0=ALU.mult, op1=ALU.mult)
                xx = gm.tile([P, T], f32)
                nc.vector.scalar_tensor_tensor(out=xx[:], in0=ux,
                                               scalar=0.5 * SQ3, in1=ux,
                                               op0=ALU.mult, op1=ALU.mult)
                yy = gm.tile([P, T], f32)
                nc.vector.scalar_tensor_tensor(out=yy[:], in0=uy,
                                               scalar=0.5 * SQ3, in1=uy,
                                               op0=ALU.mult, op1=ALU.mult)
                nc.vector.tensor_tensor(out=sh[:, 7, :], in0=xx[:],
                                        in1=yy[:], op=ALU.subtract)

                ev = gm.tile([P, N_BASIS, T], f32)
                nc.vector.tensor_tensor(
                    out=ev[:],
                    in0=dd[:].unsqueeze(1).to_broadcast([P, N_BASIS, T]),
                    in1=mu_sb[:].unsqueeze(2).to_broadcast([P, N_BASIS, T]),
                    op=ALU.subtract)
                e2 = gm.tile([P, N_BASIS, T], f32)
                nc.vector.tensor_tensor(out=e2[:], in0=ev[:], in1=ev[:],
                                        op=ALU.mult)
                gauss = gm.tile([P, N_BASIS, T], f32)
                nc.scalar.activation(gauss[:], e2[:], AF.Exp,
                                     bias=b_zero[:],
                                     scale=-1.0 / (SIGMA * SIGMA))
                tcv = gm.tile([P, T], f32)
                nc.vector.tensor_scalar(out=tcv[:], in0=dd[:],
                                        scalar1=CUTOFF - CUTOFF_WIDTH,
                                        scalar2=1.0 / CUTOFF_WIDTH,
                                        op0=ALU.subtract, op1=ALU.mult)
                nc.vector.tensor_scalar(out=tcv[:], in0=tcv[:],
                                        scalar1=0.0, scalar2=1.0,
                                        op0=ALU.max, op1=ALU.min)
                cosv = gm.tile([P, T], f32)
                nc.scalar.activation(cosv[:], tcv[:], AF.Sin,
                                     bias=b_half_pi[:],
                                     scale=-float(np.pi))
                fc = gm.tile([P, T], f32)
                nc.vector.tensor_scalar(out=fc[:], in0=cosv[:],
                                        scalar1=0.5, scalar2=0.5,
                                        op0=ALU.mult, op1=ALU.add)
                rb = gm.tile([P, N_BASIS, T], f32)
                nc.vector.tensor_tensor(
                    out=rb[:], in0=gauss[:],
                    in1=fc[:].unsqueeze(1).to_broadcast([P, N_BASIS, T]),
                    op=ALU.mult)

                # Phase C: chunked V/S build + scatter matmuls
                with tc.tile_pool(name="pair", bufs=2) as wp:
                    for ch in range(NCH):
                        t0 = ch * TC
                        vt = wp.tile([P, TC, 72], r_sc)
                        nc.vector.tensor_copy(
                            vt[:, :, 0:8],
                            rb[:, :, t0:t0 + TC].rearrange("p b t -> p t b"))
                        nc.vector.tensor_tensor(
                            out=vt[:, :, 8:72].rearrange(
                                "p t (lm b) -> p t lm b", lm=8, b=8),
                            in0=sh[:, :, t0:t0 + TC]
                                .rearrange("p lm t -> p t lm")
                                .unsqueeze(3).to_broadcast([P, TC, 8, 8]),
                            in1=rb[:, :, t0:t0 + TC]
                                .rearrange("p b t -> p t b")
                                .unsqueeze(2).to_broadcast([P, TC, 8, 8]),
                            op=ALU.mult)
                        st = wp.tile([P, TC, P], r_sc)
                        nc.vector.tensor_tensor(
                            out=st[:],
                            in0=colT[:, t0:t0 + TC].unsqueeze(2)
                                .to_broadcast([P, TC, P]),
                            in1=iota_sb[:].unsqueeze(1)
                                .to_broadcast([P, TC, P]),
                            op=ALU.is_equal)
                        for bl in range(BPC):
                            b = ch * BPC + bl
                            psg = pp.tile([72, P], f32, space="PSUM",
                                          tag="psG")
                            for j in range(TPB):
                                tt = bl * TPB + j
                                nc.tensor.matmul(out=psg[:],
                                                 lhsT=vt[:, tt, :],
                                                 rhs=st[:, tt, :],
                                                 start=(j == 0),
                                                 stop=(j == TPB - 1))
                            nc.scalar.copy(g_sb[:, b * P:(b + 1) * P], psg[:])

            # ================= atom stage =================
            with tc.tile_pool(name="atom", bufs=1) as ap:
                # f matmuls: fT[k, lm, slot]
                g4 = g_sb[:].rearrange("p (blk a s) -> p blk a s",
                                       a=A_BLK, s=N_TYPES)
                csizes = []
                c0 = 0
                while c0 < NBLK:
                    cb = min(16, NBLK - c0)
                    csizes.append((c0, cb))
                    c0 += cb
                ft = ap.tile([K, 9, NS], r_cg)
                for lm in range(9):
                    for (cb0, cbn) in csizes:
                        psf = pp.tile([K, 512], f32, space="PSUM", tag="ps512")
                        n = cbn * A_BLK
                        for s in range(N_TYPES):
                            nc.tensor.matmul(
                                out=psf[:, 0:n],
                                lhsT=mcol_sb[:, (lm * 4 + s) * K:(lm * 4 + s + 1) * K],
                                rhs=g4[:, cb0:cb0 + cbn, :, s],
                                start=(s == 0), stop=(s == N_TYPES - 1))
                        nc.scalar.copy(ft[:, lm, cb0 * A_BLK:cb0 * A_BLK + n],
                                       psf[:, 0:n])

                # CG stage: tl[k, l, slot]
                tl = ap.tile([K, 3, NS], f32)
                tmp = ap.tile([K, 512], f32)
                for l in range(3):
                    lms = [i for i in range(9) if L_OF_LM[i] == l]
                    for (cb0, cbn) in csizes:
                        n = cbn * A_BLK
                        sl = slice(cb0 * A_BLK, cb0 * A_BLK + n)
                        for mi, lm in enumerate(lms):
                            psc = pp.tile([K, 512], f32, space="PSUM",
                                          tag="ps512")
                            nc.tensor.matmul(
                                out=psc[:, 0:n],
                                lhsT=wcg_sb[:, l * K:(l + 1) * K],
                                rhs=ft[:, lm, sl],
                                start=True, stop=True)
                            if mi == 0:
                                nc.vector.tensor_tensor(
                                    out=tl[:, l, sl], in0=psc[:, 0:n],
                                    in1=ft[:, lm, sl], op=ALU.mult)
                            else:
                                nc.vector.tensor_tensor(
                                    out=tmp[:, 0:n], in0=psc[:, 0:n],
                                    in1=ft[:, lm, sl], op=ALU.mult)
                                nc.vector.tensor_tensor(
                                    out=tl[:, l, sl], in0=tl[:, l, sl],
                                    in1=tmp[:, 0:n], op=ALU.add)
                        if l == 0:
                            nc.vector.tensor_tensor(
                                out=tl[:, 0, sl], in0=tl[:, 0, sl],
                                in1=ft[:, 0, sl], op=ALU.add)

                # species one-hot of centers and embedding factor
                oct_sb = ap.tile([N_TYPES, NS], f32)
                nc.vector.tensor_tensor(
                    out=oct_sb[:], in0=specr_sb[:],
                    in1=svals_sb[:].to_broadcast([N_TYPES, NS]),
                    op=ALU.is_equal)
                x0e = ap.tile([K, 3, NS], r_hd)
                for l in range(3):
                    for (cb0, cbn) in csizes:
                        n = cbn * A_BLK
                        sl = slice(cb0 * A_BLK, cb0 * A_BLK + n)
                        pse = pp.tile([K, 512], f32, space="PSUM", tag="ps512")
                        nc.tensor.matmul(out=pse[:, 0:n],
                                         lhsT=eexp_sb[:, l * K:(l + 1) * K],
                                         rhs=oct_sb[:, sl],
                                         start=True, stop=True)
                        nc.vector.tensor_tensor(out=x0e[:, l, sl],
                                                in0=pse[:, 0:n],
                                                in1=tl[:, l, sl], op=ALU.mult)

                # head: hT[j, slot] = silu(sum_R Whead[R, j] x0e[R, slot] + b)
                ht = ap.tile([K, 3, NS], r_hd)
                for jc in range(3):
                    for (cb0, cbn) in csizes:
                        n = cbn * A_BLK
                        sl = slice(cb0 * A_BLK, cb0 * A_BLK + n)
                        psh = pp.tile([K, 512], f32, space="PSUM", tag="ps512")
                        for rc in range(3):
                            nc.tensor.matmul(
                                out=psh[:, 0:n],
                                lhsT=whead_sb[rc][:, jc * K:(jc + 1) * K],
                                rhs=x0e[:, rc, sl],
                                start=(rc == 0), stop=(rc == 2))
                        nc.scalar.activation(ht[:, jc, sl], psh[:, 0:n],
                                             AF.Silu,
                                             bias=bhead_sb[:, jc:jc + 1],
                                             scale=1.0)

                # out row
                outsb = ap.tile([1, NS], f32)
                for (cb0, cbn) in csizes:
                    n = cbn * A_BLK
                    sl = slice(cb0 * A_BLK, cb0 * A_BLK + n)
                    pso = pp.tile([1, 512], f32, space="PSUM", tag="psO")
                    for rc in range(3):
                        nc.tensor.matmul(out=pso[:, 0:n],
                                         lhsT=wout_sb[:, rc:rc + 1],
                                         rhs=ht[:, rc, sl],
                                         start=(rc == 0), stop=(rc == 2))
                    nc.scalar.activation(outsb[:, sl], pso[:, 0:n],
                                         AF.Identity,
                                         bias=bout_sb[:], scale=1.0)
                nc.sync.dma_start(out_d.ap(), outsb[:])

    nc.compile()
    return nc, T


def _prep_inputs(inputs, TPB):
    """Host-side sharding: sort pairs by center, bucket into per-core,
    per-block tile slots, and materialize per-pair endpoint positions."""
    T = NBLK * TPB
    pos = np.ascontiguousarray(np.asarray(inputs["positions"], np.float32))
    spec = np.asarray(inputs["species"]).astype(np.int64)
    pairs = np.asarray(inputs["pairs"]).astype(np.int64)
    ctr, nbr = pairs[:, 0], pairs[:, 1]
    order = np.argsort(ctr, kind="stable")
    ctr = ctr[order]
    nbr = nbr[order]
    spec_nb = spec[nbr].astype(np.float32)

    core = ctr // NLOC
    loc = ctr - core * NLOC
    blk = loc // A_BLK
    arel = loc - blk * A_BLK

    # rank within (core, block)
    key = core * NBLK + blk
    # pairs sorted by ctr -> key is non-decreasing
    counts = np.bincount(key, minlength=NCORES * NBLK)
    starts = np.concatenate([[0], np.cumsum(counts)[:-1]])
    rank = np.arange(len(ctr)) - starts[key]

    slot = blk * (TPB * P) + rank          # slot within core's pair arrays
    tt = slot // P
    qq = slot - tt * P

    in_maps = []
    # constant tables (shared across cores)
    iota_np = np.broadcast_to(np.arange(P, dtype=np.float32), (P, P)).copy()
    mu_np = np.broadcast_to(
        np.linspace(0.0, CUTOFF, N_BASIS, dtype=np.float32), (P, N_BASIS)).copy()

    emb = np.asarray(inputs["embeddings"], np.float32)
    h0t = np.repeat(emb, N_MAX, axis=1)                    # [4, 128]
    W_rad = np.asarray(inputs["W_rad"], np.float32)
    mcol = np.zeros((72, 36 * K), np.float32)
    for lm in range(9):
        l = L_OF_LM[lm]
        for s in range(N_TYPES):
            blkc = (lm * 4 + s) * K
            for b in range(N_BASIS):
                mcol[lm * 8 + b, blkc:blkc + K] = \
                    MP_SCALING * W_rad[l, b, :] * h0t[s, :]
    wcg = np.concatenate([
        np.asarray(inputs["W_cg0"], np.float32),
        np.asarray(inputs["W_cg1"], np.float32) * np.float32(-1.0 / SQ3),
        np.asarray(inputs["W_cg2"], np.float32) * np.float32(1.0 / SQ3),
    ], axis=1)                                             # [128, 384]
    eexp = np.repeat(emb, K0_TOT // N_CHANNELS, axis=1)    # [4, 384]
    W_head = np.asarray(inputs["W_head"], np.float32)      # [384, 384]
    whead = np.stack([W_head[i * K:(i + 1) * K, :] for i in range(3)])
    b_head = np.asarray(inputs["b_head"], np.float32)
    bhead = b_head.reshape(3, K).T.copy()                  # [128, 3]
    W_out = np.asarray(inputs["W_out"], np.float32)        # [384, 1]
    wout = W_out[:, 0].reshape(3, K).T.copy()              # [128, 3]
    bout = np.asarray(inputs["b_out"], np.float32).reshape(1, 1)

    for c in range(NCORES):
        m = core == c
        posnb = np.zeros((P, T, 3), np.float32)
        posct = np.zeros((P, T, 3), np.float32)
        colf = np.full((P, T), -1.0, np.float32)
        posnb[qq[m], tt[m]] = pos[nbr[m]]
        posct[qq[m], tt[m]] = pos[ctr[m]]
        colf[qq[m], tt[m]] = arel[m] * N_TYPES + spec_nb[m]
        slots = np.arange(NS)
        atom = c * NLOC + np.minimum(slots, NLOC - 1)
        specr = np.broadcast_to(spec[atom].astype(np.float32), (N_TYPES, NS)).copy()
        in_maps.append(dict(
            posnb=posnb, posct=posct, colf=colf, specr=specr,
            iota=iota_np, mu=mu_np, mcol=mcol, wcg=wcg, eexp=eexp,
            whead=whead, bhead=bhead, wout=wout, bout=bout,
            svals=np.arange(N_TYPES, dtype=np.float32).reshape(N_TYPES, 1),
        ))
    return in_maps


def _required_tpb(inputs):
    pairs = np.asarray(inputs["pairs"]).astype(np.int64)
    ctr = pairs[:, 0]
    key = (ctr // NLOC) * NBLK + (ctr % NLOC) // A_BLK
    counts = np.bincount(key, minlength=NCORES * NBLK)
    return max(5, int(math.ceil(counts.max() / P)))


def _install_ntff_hook():
    """Provide the antenv.axon_hooks registry this image lacks, backed by
    direct ctypes calls into libaxon_pjrt.so (same mechanism trn_boot uses)."""
    import types
    if "antenv.axon_hooks" in sys.modules:
        return
    try:
        import antenv
        from trn_agent_boot.trn_boot import _ntff_profile_via_ctypes
        hook = _ntff_profile_via_ctypes("/opt/axon/libaxon_pjrt.so")
        mod = types.ModuleType("antenv.axon_hooks")
        _h = {"hook": hook}
        mod.get_axon_ntff_profile_hook = lambda: _h["hook"]
        mod.set_axon_ntff_profile_hook = lambda h: _h.__setitem__("hook", h)
        sys.modules["antenv.axon_hooks"] = mod
        antenv.axon_hooks = mod
        bass_utils.upload_artifacts = lambda d: f"file://{d}"
    except Exception as e:
        print("ntff hook install failed:", repr(e))


def run_cores(inputs, trace=False):
    if trace:
        _install_ntff_hook()
    TPB = _required_tpb(inputs)
    if TPB not in _BUILD_CACHE:
        _BUILD_CACHE[TPB] = _build(TPB)
    nc, T = _BUILD_CACHE[TPB]
    in_maps = _prep_inputs(inputs, TPB)
    res = bass_utils.run_bass_kernel_spmd(
        nc, in_maps, core_ids=list(range(NCORES)), trace=trace)
    outs = [res.results[c]["out"][0, :NLOC] for c in range(NCORES)]
    full = np.concatenate(outs).reshape(N_ATOMS, 1).astype(np.float32)
    return full, res


def kernel(**inputs):
    full, _ = run_cores(inputs, trace=False)
    return full


# revision 7
# speedup vs baseline: 1.7494x; 1.0707x over previous
"""Trainium2 Bass kernel for nn_BaseModel_2654289789315 (gnn_message_passing).

Strategy (validated numerically in fp64/fp32 on CPU):
  - The reference network's output depends only on the L=0 invariant channel.
    The L=1/L=2 uncoupled matrices are antisymmetric / traceless-symmetric, so
    the whole model reduces to per-(l,m) vectors f[atom, lm, 128] and traces:
        t_0 = (f0 @ W0) * f0 + f0
        t_l = s_l/sqrt(3) * sum_m (f_lm @ W_l) * f_lm   (s_1=-1, s_2=+1)
  - neigh features depend only on the neighbor's species (4 values) and
    R_l = rb @ W_rad, so the message-passing segment-sum only needs
        G[atom, lm, basis(8), species(4)]  (288 scalars per atom),
    computed on-device as a one-hot matmul scatter:
        G_block = sum_tiles V^T @ S   with V[pair,72]=sh x rb (outer product),
        S[pair,128] one-hot of (atom_in_block*4 + neighbor_species).
  - All 128-channel work happens in small dense per-atom matmuls.

Sharding: atoms (and their incident pairs, grouped by center) are sharded
across 8 cores; small weights are replicated; no collectives are needed
because each core owns all pairs of its atoms (neighbor data is materialized
per-shard on the host, i.e. the "halo exchange" happens at input-marshaling
time).
"""

import sys
if "/opt/trn_rl_repo" not in sys.path:
    sys.path.insert(0, "/opt/trn_rl_repo")

import math
import numpy as np

import concourse.bass as bass
import concourse.mybir as mybir
import concourse.tile as tile
from concourse import bacc, bass_utils

AF = mybir.ActivationFunctionType
ALU = mybir.AluOpType
DT = mybir.dt

# ---- problem constants (hardcoded per task spec) ----
N_ATOMS = 10000
N_PAIRS = 160000
N_TYPES = 4
N_CHANNELS = 32
N_MAX = 4
N_BASIS = 8
K = 128
L_MAX = 2
CUTOFF = 20.0
CUTOFF_WIDTH = 5.0
MP_SCALING = 0.1
K0_TOT = 384
NCORES = 8
NLOC = N_ATOMS // NCORES          # 1250 atoms per core
A_BLK = 32                         # atoms per scatter block
NBLK = math.ceil(NLOC / A_BLK)     # 40
NS = NBLK * A_BLK                  # 1280 output slots per core
P = 128
SQ3 = float(np.sqrt(3.0))
SIGMA = CUTOFF / N_BASIS           # 2.5
L_OF_LM = [0, 1, 1, 1, 2, 2, 2, 2, 2]

# dtype config: stage-wise float32r (PE fast path, ~1e-4 relative rounding)
F16_SCATTER = True
F32R_F = True
F32R_CG = True
F32R_HEAD = True

_BUILD_CACHE = {}


def _build(TPB):
    """Build + compile the single-core Bass program (SPMD across 8 cores)."""
    T = NBLK * TPB                # total pair tiles
    BPC = 8                       # blocks per pair-stage chunk
    NCH = NBLK // BPC             # 5 chunks
    TC = BPC * TPB                # tiles per chunk

    nc = bacc.Bacc("TRN2", target_bir_lowering=False, debug=False,
                   num_devices=NCORES)

    def din(name, shape, dt=DT.float32):
        return nc.dram_tensor(name, shape, dt, kind="ExternalInput")

    posnb_d = din("posnb", [P, T, 3])
    posct_d = din("posct", [P, T, 3])
    colf_d = din("colf", [P, T], DT.float16)
    specr_d = din("specr", [N_TYPES, NS])
    iota16_d = din("iota16", [P, P], DT.float16)
    iota_d = din("iota", [P, P])
    mu_d = din("mu", [P, N_BASIS])
    mcol_d = din("mcol", [72, 36 * K])
    wcg_d = din("wcg", [K, 3 * K])
    eexp_d = din("eexp", [N_TYPES, K0_TOT])
    whead_d = din("whead", [3, K, K0_TOT])
    bhead_d = din("bhead", [K, 3])
    wout_d = din("wout", [K, 3])
    bout_d = din("bout", [1, 1])
    svals_d = din("svals", [N_TYPES, 1])
    out_d = nc.dram_tensor("out", [1, NS], DT.float32, kind="ExternalOutput")

    f32 = DT.float32
    r_sc = DT.float16 if F16_SCATTER else f32
    r_f = DT.float32r if F32R_F else f32
    r_cg = DT.float32r if F32R_CG else f32
    r_hd = DT.float32r if F32R_HEAD else f32

    with tile.TileContext(nc) as tc:
        with tc.tile_pool(name="const", bufs=1) as cp, \
             tc.tile_pool(name="gpool", bufs=1) as gp, \
             tc.tile_pool(name="psum", bufs=2, space="PSUM") as pp:

            # ---- constants into SBUF ----
            iota_sb = cp.tile([P, P], f32)
            nc.sync.dma_start(iota_sb[:], iota_d.ap())
            iota16_sb = cp.tile([P, P], DT.float16)
            nc.sync.dma_start(iota16_sb[:], iota16_d.ap())
            mu_sb = cp.tile([P, N_BASIS], f32)
            nc.sync.dma_start(mu_sb[:], mu_d.ap())
            mcol_sb = cp.tile([72, 36 * K], r_f)
            if F32R_F:
                mcol_f32 = cp.tile([72, 36 * K], f32)
                nc.sync.dma_start(mcol_f32[:], mcol_d.ap())
                nc.vector.tensor_copy(mcol_sb[:], mcol_f32[:])
            else:
                nc.sync.dma_start(mcol_sb[:], mcol_d.ap())
            wcg_sb = cp.tile([K, 3 * K], r_cg)
            if F32R_CG:
                wcg_f32 = cp.tile([K, 3 * K], f32)
                nc.sync.dma_start(wcg_f32[:], wcg_d.ap())
                nc.vector.tensor_copy(wcg_sb[:], wcg_f32[:])
            else:
                nc.sync.dma_start(wcg_sb[:], wcg_d.ap())
            eexp_sb = cp.tile([N_TYPES, K0_TOT], f32)
            nc.sync.dma_start(eexp_sb[:], eexp_d.ap())
            whead_sb = [cp.tile([K, K0_TOT], r_hd, name=f"whead{i}", tag=f"whead{i}") for i in range(3)]
            for i in range(3):
                if F32R_HEAD:
                    wtmp = cp.tile([K, K0_TOT], f32, tag=f"wheadf{i}")
                    nc.sync.dma_start(wtmp[:], whead_d.ap()[i])
                    nc.vector.tensor_copy(whead_sb[i][:], wtmp[:])
                else:
                    nc.sync.dma_start(whead_sb[i][:], whead_d.ap()[i])
            bhead_sb = cp.tile([K, 3], f32)
            nc.sync.dma_start(bhead_sb[:], bhead_d.ap())
            wout_sb = cp.tile([K, 3], r_hd)
            if F32R_HEAD:
                wout_f32 = cp.tile([K, 3], f32)
                nc.sync.dma_start(wout_f32[:], wout_d.ap())
                nc.vector.tensor_copy(wout_sb[:], wout_f32[:])
            else:
                nc.sync.dma_start(wout_sb[:], wout_d.ap())
            bout_sb = cp.tile([1, 1], f32)
            nc.sync.dma_start(bout_sb[:], bout_d.ap())
            specr_sb = cp.tile([N_TYPES, NS], f32)
            nc.sync.dma_start(specr_sb[:], specr_d.ap())
            svals_sb = cp.tile([N_TYPES, 1], f32)
            nc.sync.dma_start(svals_sb[:], svals_d.ap())

            def bias_tile(val, tag):
                bt = cp.tile([P, 1], f32, tag=tag)
                nc.vector.memset(bt[:], val)
                return bt

            b_eps = bias_tile(1e-12, "b_eps")
            b_half_pi = bias_tile(float(np.pi / 2), "b_hpi")
            b_zero = bias_tile(0.0, "b_zero")

            # ---- G accumulator in SBUF ----
            g_sb = gp.tile([72, NBLK * P], r_f)

            # ================= pair stage =================
            # Phase A/B: whole-T geometry + activations batched by table set
            # (Ln/Exp together, then Sin) to avoid ACT table thrash.
            with tc.tile_pool(name="geom", bufs=1) as gm:
                pnb = gm.tile([P, T, 3], f32)
                nc.sync.dma_start(pnb[:], posnb_d.ap())
                pct = gm.tile([P, T, 3], f32)
                nc.sync.dma_start(pct[:], posct_d.ap())
                colT = gm.tile([P, T], DT.float16)
                nc.sync.dma_start(colT[:], colf_d.ap())

                rv = gm.tile([P, T, 3], f32)
                nc.vector.tensor_tensor(out=rv[:], in0=pnb[:], in1=pct[:],
                                        op=ALU.subtract)
                rr = gm.tile([P, T], f32)
                nc.vector.tensor_tensor(out=rr[:], in0=rv[:, :, 0],
                                        in1=rv[:, :, 0], op=ALU.mult)
                tmp2 = gm.tile([P, T], f32)
                nc.vector.tensor_tensor(out=tmp2[:], in0=rv[:, :, 1],
                                        in1=rv[:, :, 1], op=ALU.mult)
                nc.vector.tensor_tensor(out=rr[:], in0=rr[:], in1=tmp2[:],
                                        op=ALU.add)
                nc.vector.tensor_tensor(out=tmp2[:], in0=rv[:, :, 2],
                                        in1=rv[:, :, 2], op=ALU.mult)
                nc.vector.tensor_tensor(out=rr[:], in0=rr[:], in1=tmp2[:],
                                        op=ALU.add)
                lnrr = gm.tile([P, T], f32)
                nc.scalar.activation(lnrr[:], rr[:], AF.Ln,
                                     bias=b_eps[:], scale=1.0)
                dd = gm.tile([P, T], f32)
                nc.scalar.activation(dd[:], lnrr[:], AF.Exp,
                                     bias=b_zero[:], scale=0.5)
                invd = gm.tile([P, T], f32)
                nc.scalar.activation(invd[:], lnrr[:], AF.Exp,
                                     bias=b_zero[:], scale=-0.5)
                uv = gm.tile([P, T, 3], f32)
                nc.vector.tensor_tensor(
                    out=uv[:], in0=rv[:],
                    in1=invd[:].unsqueeze(2).to_broadcast([P, T, 3]),
                    op=ALU.mult)

                sh = gm.tile([P, 8, T], f32)
                ux, uy, uz = uv[:, :, 0], uv[:, :, 1], uv[:, :, 2]
                nc.vector.tensor_copy(sh[:, 0, :], uy)
                nc.vector.tensor_copy(sh[:, 1, :], uz)
                nc.vector.tensor_copy(sh[:, 2, :], ux)
                nc.vector.scalar_tensor_tensor(out=sh[:, 3, :], in0=ux,
                                               scalar=SQ3, in1=uy,
                                               op0=ALU.mult, op1=ALU.mult)
                nc.vector.scalar_tensor_tensor(out=sh[:, 4, :], in0=uy,
                                               scalar=SQ3, in1=uz,
                                               op0=ALU.mult, op1=ALU.mult)
                zz3 = gm.tile([P, T], f32)
                nc.vector.scalar_tensor_tensor(out=zz3[:], in0=uz,
                                               scalar=3.0, in1=uz,
                                               op0=ALU.mult, op1=ALU.mult)
                nc.vector.tensor_scalar(out=sh[:, 5, :], in0=zz3[:],
                                        scalar1=0.5, scalar2=-0.5,
                                        op0=ALU.mult, op1=ALU.add)
                nc.vector.scalar_tensor_tensor(out=sh[:, 6, :], in0=ux,
                                               scalar=SQ3, in1=uz,
                                               op0=ALU.mult, op1=ALU.mult)
                xx = gm.tile([P, T], f32)
                nc.vector.scalar_tensor_tensor(out=xx[:], in0=ux,
                                               scalar=0.5 * SQ3, in1=ux,
                                               op0=ALU.mult, op1=ALU.mult)
                yy = gm.tile([P, T], f32)
                nc.vector.scalar_tensor_tensor(out=yy[:], in0=uy,
                                               scalar=0.5 * SQ3, in1=uy,
                                               op0=ALU.mult, op1=ALU.mult)
                nc.vector.tensor_tensor(out=sh[:, 7, :], in0=xx[:],
                                        in1=yy[:], op=ALU.subtract)

                ev = gm.tile([P, N_BASIS, T], f32)
                nc.vector.tensor_tensor(
                    out=ev[:],
                    in0=dd[:].unsqueeze(1).to_broadcast([P, N_BASIS, T]),
                    in1=mu_sb[:].unsqueeze(2).to_broadcast([P, N_BASIS, T]),
                    op=ALU.subtract)
                e2 = gm.tile([P, N_BASIS, T], f32)
                nc.vector.tensor_tensor(out=e2[:], in0=ev[:], in1=ev[:],
                                        op=ALU.mult)
                gauss = gm.tile([P, N_BASIS, T], f32)
                nc.scalar.activation(gauss[:], e2[:], AF.Exp,
                                     bias=b_zero[:],
                                     scale=-1.0 / (SIGMA * SIGMA))
                tcv = gm.tile([P, T], f32)
                nc.vector.tensor_scalar(out=tcv[:], in0=dd[:],
                                        scalar1=CUTOFF - CUTOFF_WIDTH,
                                        scalar2=1.0 / CUTOFF_WIDTH,
                                        op0=ALU.subtract, op1=ALU.mult)
                nc.vector.tensor_scalar(out=tcv[:], in0=tcv[:],
                                        scalar1=0.0, scalar2=1.0,
                                        op0=ALU.max, op1=ALU.min)
                cosv = gm.tile([P, T], f32)
                nc.scalar.activation(cosv[:], tcv[:], AF.Sin,
                                     bias=b_half_pi[:],
                                     scale=-float(np.pi))
                fc = gm.tile([P, T], f32)
                nc.vector.tensor_scalar(out=fc[:], in0=cosv[:],
                                        scalar1=0.5, scalar2=0.5,
                                        op0=ALU.mult, op1=ALU.add)
                rb = gm.tile([P, N_BASIS, T], f32)
                nc.vector.tensor_tensor(
                    out=rb[:], in0=gauss[:],
                    in1=fc[:].unsqueeze(1).to_broadcast([P, N_BASIS, T]),
                    op=ALU.mult)

                # Phase C: chunked V/S build + scatter matmuls
                with tc.tile_pool(name="pair", bufs=2) as wp:
                    for ch in range(NCH):
                        t0 = ch * TC
                        vt = wp.tile([P, TC, 72], r_sc)
                        nc.vector.tensor_copy(
                            vt[:, :, 0:8],
                            rb[:, :, t0:t0 + TC].rearrange("p b t -> p t b"))
                        nc.vector.tensor_tensor(
                            out=vt[:, :, 8:72].rearrange(
                                "p t (lm b) -> p t lm b", lm=8, b=8),
                            in0=sh[:, :, t0:t0 + TC]
                                .rearrange("p lm t -> p t lm")
                                .unsqueeze(3).to_broadcast([P, TC, 8, 8]),
                            in1=rb[:, :, t0:t0 + TC]
                                .rearrange("p b t -> p t b")
                                .unsqueeze(2).to_broadcast([P, TC, 8, 8]),
                            op=ALU.mult)
                        st = wp.tile([P, TC, P], r_sc)
                        nc.vector.tensor_tensor(
                            out=st[:],
                            in0=colT[:, t0:t0 + TC].unsqueeze(2)
                                .to_broadcast([P, TC, P]),
                            in1=iota16_sb[:].unsqueeze(1)
                                .to_broadcast([P, TC, P]),
                            op=ALU.is_equal)
                        for bl in range(BPC):
                            b = ch * BPC + bl
                            psg = pp.tile([72, P], f32, space="PSUM",
                                          tag="psG")
                            for j in range(TPB):
                                tt = bl * TPB + j
                                nc.tensor.matmul(out=psg[:],
                                                 lhsT=vt[:, tt, :],
                                                 rhs=st[:, tt, :],
                                                 start=(j == 0),
                                                 stop=(j == TPB - 1))
                            nc.scalar.copy(g_sb[:, b * P:(b + 1) * P], psg[:])

            # ================= atom stage =================
            with tc.tile_pool(name="atom", bufs=1) as ap:
                # f matmuls: fT[k, lm, slot]
                g4 = g_sb[:].rearrange("p (blk a s) -> p blk a s",
                                       a=A_BLK, s=N_TYPES)
                csizes = []
                c0 = 0
                while c0 < NBLK:
                    cb = min(16, NBLK - c0)
                    csizes.append((c0, cb))
                    c0 += cb
                ft = ap.tile([K, 9, NS], r_cg)
                for lm in range(9):
                    for (cb0, cbn) in csizes:
                        psf = pp.tile([K, 512], f32, space="PSUM", tag="ps512")
                        n = cbn * A_BLK
                        for s in range(N_TYPES):
                            nc.tensor.matmul(
                                out=psf[:, 0:n],
                                lhsT=mcol_sb[:, (lm * 4 + s) * K:(lm * 4 + s + 1) * K],
                                rhs=g4[:, cb0:cb0 + cbn, :, s],
                                start=(s == 0), stop=(s == N_TYPES - 1))
                        nc.scalar.copy(ft[:, lm, cb0 * A_BLK:cb0 * A_BLK + n],
                                       psf[:, 0:n])

                # CG stage: tl[k, l, slot]
                tl = ap.tile([K, 3, NS], f32)
                tmp = ap.tile([K, 512], f32)
                for l in range(3):
                    lms = [i for i in range(9) if L_OF_LM[i] == l]
                    for (cb0, cbn) in csizes:
                        n = cbn * A_BLK
                        sl = slice(cb0 * A_BLK, cb0 * A_BLK + n)
                        for mi, lm in enumerate(lms):
                            psc = pp.tile([K, 512], f32, space="PSUM",
                                          tag="ps512")
                            nc.tensor.matmul(
                                out=psc[:, 0:n],
                                lhsT=wcg_sb[:, l * K:(l + 1) * K],
                                rhs=ft[:, lm, sl],
                                start=True, stop=True)
                            if mi == 0:
                                nc.vector.tensor_tensor(
                                    out=tl[:, l, sl], in0=psc[:, 0:n],
                                    in1=ft[:, lm, sl], op=ALU.mult)
                            else:
                                nc.vector.tensor_tensor(
                                    out=tmp[:, 0:n], in0=psc[:, 0:n],
                                    in1=ft[:, lm, sl], op=ALU.mult)
                                nc.vector.tensor_tensor(
                                    out=tl[:, l, sl], in0=tl[:, l, sl],
                                    in1=tmp[:, 0:n], op=ALU.add)
                        if l == 0:
                            nc.vector.tensor_tensor(
                                out=tl[:, 0, sl], in0=tl[:, 0, sl],
                                in1=ft[:, 0, sl], op=ALU.add)

                # species one-hot of centers and embedding factor
                oct_sb = ap.tile([N_TYPES, NS], f32)
                nc.vector.tensor_tensor(
                    out=oct_sb[:], in0=specr_sb[:],
                    in1=svals_sb[:].to_broadcast([N_TYPES, NS]),
                    op=ALU.is_equal)
                x0e = ap.tile([K, 3, NS], r_hd)
                for l in range(3):
                    for (cb0, cbn) in csizes:
                        n = cbn * A_BLK
                        sl = slice(cb0 * A_BLK, cb0 * A_BLK + n)
                        pse = pp.tile([K, 512], f32, space="PSUM", tag="ps512")
                        nc.tensor.matmul(out=pse[:, 0:n],
                                         lhsT=eexp_sb[:, l * K:(l + 1) * K],
                                         rhs=oct_sb[:, sl],
                                         start=True, stop=True)
                        nc.vector.tensor_tensor(out=x0e[:, l, sl],
                                                in0=pse[:, 0:n],
                                                in1=tl[:, l, sl], op=ALU.mult)

                # head: hT[j, slot] = silu(sum_R Whead[R, j] x0e[R, slot] + b)
                ht = ap.tile([K, 3, NS], r_hd)
                for jc in range(3):
                    for (cb0, cbn) in csizes:
                        n = cbn * A_BLK
                        sl = slice(cb0 * A_BLK, cb0 * A_BLK + n)
                        psh = pp.tile([K, 512], f32, space="PSUM", tag="ps512")
                        for rc in range(3):
                            nc.tensor.matmul(
                                out=psh[:, 0:n],
                                lhsT=whead_sb[rc][:, jc * K:(jc + 1) * K],
                                rhs=x0e[:, rc, sl],
                                start=(rc == 0), stop=(rc == 2))
                        nc.scalar.activation(ht[:, jc, sl], psh[:, 0:n],
                                             AF.Silu,
                                             bias=bhead_sb[:, jc:jc + 1],
                                             scale=1.0)

                # out row
                outsb = ap.tile([1, NS], f32)
                for (cb0, cbn) in csizes:
                    n = cbn * A_BLK
                    sl = slice(cb0 * A_BLK, cb0 * A_BLK + n)
                    pso = pp.tile([1, 512], f32, space="PSUM", tag="psO")
                    for rc in range(3):
                        nc.tensor.matmul(out=pso[:, 0:n],
                                         lhsT=wout_sb[:, rc:rc + 1],
                                         rhs=ht[:, rc, sl],
                                         start=(rc == 0), stop=(rc == 2))
                    nc.scalar.activation(outsb[:, sl], pso[:, 0:n],
                                         AF.Identity,
                                         bias=bout_sb[:], scale=1.0)
                nc.sync.dma_start(out_d.ap(), outsb[:])

    nc.compile()
    return nc, T


def _prep_inputs(inputs, TPB):
    """Host-side sharding: sort pairs by center, bucket into per-core,
    per-block tile slots, and materialize per-pair endpoint positions."""
    T = NBLK * TPB
    pos = np.ascontiguousarray(np.asarray(inputs["positions"], np.float32))
    spec = np.asarray(inputs["species"]).astype(np.int64)
    pairs = np.asarray(inputs["pairs"]).astype(np.int64)
    ctr, nbr = pairs[:, 0], pairs[:, 1]
    order = np.argsort(ctr, kind="stable")
    ctr = ctr[order]
    nbr = nbr[order]
    spec_nb = spec[nbr].astype(np.float32)

    core = ctr // NLOC
    loc = ctr - core * NLOC
    blk = loc // A_BLK
    arel = loc - blk * A_BLK

    # rank within (core, block)
    key = core * NBLK + blk
    # pairs sorted by ctr -> key is non-decreasing
    counts = np.bincount(key, minlength=NCORES * NBLK)
    starts = np.concatenate([[0], np.cumsum(counts)[:-1]])
    rank = np.arange(len(ctr)) - starts[key]

    slot = blk * (TPB * P) + rank          # slot within core's pair arrays
    tt = slot // P
    qq = slot - tt * P

    in_maps = []
    # constant tables (shared across cores)
    iota_np = np.broadcast_to(np.arange(P, dtype=np.float32), (P, P)).copy()
    mu_np = np.broadcast_to(
        np.linspace(0.0, CUTOFF, N_BASIS, dtype=np.float32), (P, N_BASIS)).copy()

    emb = np.asarray(inputs["embeddings"], np.float32)
    h0t = np.repeat(emb, N_MAX, axis=1)                    # [4, 128]
    W_rad = np.asarray(inputs["W_rad"], np.float32)
    mcol = np.zeros((72, 36 * K), np.float32)
    for lm in range(9):
        l = L_OF_LM[lm]
        for s in range(N_TYPES):
            blkc = (lm * 4 + s) * K
            for b in range(N_BASIS):
                mcol[lm * 8 + b, blkc:blkc + K] = \
                    MP_SCALING * W_rad[l, b, :] * h0t[s, :]
    wcg = np.concatenate([
        np.asarray(inputs["W_cg0"], np.float32),
        np.asarray(inputs["W_cg1"], np.float32) * np.float32(-1.0 / SQ3),
        np.asarray(inputs["W_cg2"], np.float32) * np.float32(1.0 / SQ3),
    ], axis=1)                                             # [128, 384]
    eexp = np.repeat(emb, K0_TOT // N_CHANNELS, axis=1)    # [4, 384]
    W_head = np.asarray(inputs["W_head"], np.float32)      # [384, 384]
    whead = np.stack([W_head[i * K:(i + 1) * K, :] for i in range(3)])
    b_head = np.asarray(inputs["b_head"], np.float32)
    bhead = b_head.reshape(3, K).T.copy()                  # [128, 3]
    W_out = np.asarray(inputs["W_out"], np.float32)        # [384, 1]
    wout = W_out[:, 0].reshape(3, K).T.copy()              # [128, 3]
    bout = np.asarray(inputs["b_out"], np.float32).reshape(1, 1)

    for c in range(NCORES):
        m = core == c
        posnb = np.zeros((P, T, 3), np.float32)
        posct = np.zeros((P, T, 3), np.float32)
        colf = np.full((P, T), -1.0, np.float16)
        posnb[qq[m], tt[m]] = pos[nbr[m]]
        posct[qq[m], tt[m]] = pos[ctr[m]]
        colf[qq[m], tt[m]] = (arel[m] * N_TYPES + spec_nb[m]).astype(np.float16)
        slots = np.arange(NS)
        atom = c * NLOC + np.minimum(slots, NLOC - 1)
        specr = np.broadcast_to(spec[atom].astype(np.float32), (N_TYPES, NS)).copy()
        in_maps.append(dict(
            posnb=posnb, posct=posct, colf=colf, specr=specr,
            iota=iota_np, iota16=iota_np.astype(np.float16),
            mu=mu_np, mcol=mcol, wcg=wcg, eexp=eexp,
            whead=whead, bhead=bhead, wout=wout, bout=bout,
            svals=np.arange(N_TYPES, dtype=np.float32).reshape(N_TYPES, 1),
        ))
    return in_maps


def _required_tpb(inputs):
    pairs = np.asarray(inputs["pairs"]).astype(np.int64)
    ctr = pairs[:, 0]
    key = (ctr // NLOC) * NBLK + (ctr % NLOC) // A_BLK
    counts = np.bincount(key, minlength=NCORES * NBLK)
    return max(5, int(math.ceil(counts.max() / P)))


def _install_ntff_hook():
    """Provide the antenv.axon_hooks registry this image lacks, backed by
    direct ctypes calls into libaxon_pjrt.so (same mechanism trn_boot uses)."""
    import types
    if "antenv.axon_hooks" in sys.modules:
        return
    try:
        import antenv
        from trn_agent_boot.trn_boot import _ntff_profile_via_ctypes
        hook = _ntff_profile_via_ctypes("/opt/axon/libaxon_pjrt.so")
        mod = types.ModuleType("antenv.axon_hooks")
        _h = {"hook": hook}
        mod.get_axon_ntff_profile_hook = lambda: _h["hook"]
        mod.set_axon_ntff_profile_hook = lambda h: _h.__setitem__("hook", h)
        sys.modules["antenv.axon_hooks"] = mod
        antenv.axon_hooks = mod
        bass_utils.upload_artifacts = lambda d: f"file://{d}"
    except Exception as e:
        print("ntff hook install failed:", repr(e))


def run_cores(inputs, trace=False):
    if trace:
        _install_ntff_hook()
    TPB = _required_tpb(inputs)
    if TPB not in _BUILD_CACHE:
        _BUILD_CACHE[TPB] = _build(TPB)
    nc, T = _BUILD_CACHE[TPB]
    in_maps = _prep_inputs(inputs, TPB)
    res = bass_utils.run_bass_kernel_spmd(
        nc, in_maps, core_ids=list(range(NCORES)), trace=trace)
    outs = [res.results[c]["out"][0, :NLOC] for c in range(NCORES)]
    full = np.concatenate(outs).reshape(N_ATOMS, 1).astype(np.float32)
    return full, res


def kernel(**inputs):
    full, _ = run_cores(inputs, trace=False)
    return full


# revision 13
# speedup vs baseline: 1.8411x; 1.0524x over previous
"""Trainium2 Bass kernel for nn_BaseModel_2654289789315 (gnn_message_passing).

Strategy (validated numerically in fp64/fp32 on CPU):
  - The reference network's output depends only on the L=0 invariant channel.
    The L=1/L=2 uncoupled matrices are antisymmetric / traceless-symmetric, so
    the whole model reduces to per-(l,m) vectors f[atom, lm, 128] and traces:
        t_0 = (f0 @ W0) * f0 + f0
        t_l = s_l/sqrt(3) * sum_m (f_lm @ W_l) * f_lm   (s_1=-1, s_2=+1)
  - neigh features depend only on the neighbor's species (4 values) and
    R_l = rb @ W_rad, so the message-passing segment-sum only needs
        G[atom, lm, basis(8), species(4)]  (288 scalars per atom),
    computed on-device as a one-hot matmul scatter:
        G_block = sum_tiles V^T @ S   with V[pair,72]=sh x rb (outer product),
        S[pair,128] one-hot of (atom_in_block*4 + neighbor_species).
  - All 128-channel work happens in small dense per-atom matmuls.

Sharding: atoms (and their incident pairs, grouped by center) are sharded
across 8 cores; small weights are replicated; no collectives are needed
because each core owns all pairs of its atoms (neighbor data is materialized
per-shard on the host, i.e. the "halo exchange" happens at input-marshaling
time).
"""

import sys
if "/opt/trn_rl_repo" not in sys.path:
    sys.path.insert(0, "/opt/trn_rl_repo")

import math
import numpy as np

import concourse.bass as bass
import concourse.mybir as mybir
import concourse.tile as tile
from concourse import bacc, bass_utils

AF = mybir.ActivationFunctionType
ALU = mybir.AluOpType
DT = mybir.dt

# ---- problem constants (hardcoded per task spec) ----
N_ATOMS = 10000
N_PAIRS = 160000
N_TYPES = 4
N_CHANNELS = 32
N_MAX = 4
N_BASIS = 8
K = 128
L_MAX = 2
CUTOFF = 20.0
CUTOFF_WIDTH = 5.0
MP_SCALING = 0.1
K0_TOT = 384
NCORES = 8
NLOC = N_ATOMS // NCORES          # 1250 atoms per core
A_BLK = 32                         # atoms per scatter block
NBLK = math.ceil(NLOC / A_BLK)     # 40
NS = NBLK * A_BLK                  # 1280 output slots per core
P = 128
SQ3 = float(np.sqrt(3.0))
SIGMA = CUTOFF / N_BASIS           # 2.5
L_OF_LM = [0, 1, 1, 1, 2, 2, 2, 2, 2]

# dtype config: stage-wise float32r (PE fast path, ~1e-4 relative rounding)
F16_SCATTER = True
F16_F = True
F16_CG = True
F16_HEAD = True

_BUILD_CACHE = {}


def _build(TPB):
    """Build + compile the single-core Bass program (SPMD across 8 cores)."""
    T = NBLK * TPB                # total pair tiles
    BPC = 8                       # blocks per pair-stage chunk
    NCH = NBLK // BPC             # 5 chunks
    TC = BPC * TPB                # tiles per chunk

    nc = bacc.Bacc("TRN2", target_bir_lowering=False, debug=False,
                   num_devices=NCORES)

    def din(name, shape, dt=DT.float32):
        return nc.dram_tensor(name, shape, dt, kind="ExternalInput")

    posnb_d = din("posnb", [P, T, 3])
    posct_d = din("posct", [P, T, 3])
    colf_d = din("colf", [P, T], DT.float16)
    specr_d = din("specr", [N_TYPES, NS])
    iota16_d = din("iota16", [P, P], DT.float16)
    iota_d = din("iota", [P, P])
    mu_d = din("mu", [P, N_BASIS])
    mcol_d = din("mcol", [72, 36 * K])
    wcg_d = din("wcg", [K, 3 * K])
    eexp_d = din("eexp", [N_TYPES, K0_TOT])
    whead_d = din("whead", [3, K, K0_TOT])
    bhead_d = din("bhead", [K, 3])
    wout_d = din("wout", [K, 3])
    bout_d = din("bout", [1, 1])
    svals_d = din("svals", [N_TYPES, 1])
    out_d = nc.dram_tensor("out", [1, NS], DT.float32, kind="ExternalOutput")

    f32 = DT.float32
    r_sc = DT.float16 if F16_SCATTER else f32
    r_f = DT.float16 if F16_F else f32
    r_cg = DT.float16 if F16_CG else f32
    r_hd = DT.float16 if F16_HEAD else f32

    with tile.TileContext(nc) as tc:
        with tc.tile_pool(name="const", bufs=1) as cp, \
             tc.tile_pool(name="gpool", bufs=1) as gp, \
             tc.tile_pool(name="psum", bufs=2, space="PSUM") as pp:

            # ---- constants into SBUF ----
            iota_sb = cp.tile([P, P], f32)
            nc.sync.dma_start(iota_sb[:], iota_d.ap())
            iota16_sb = cp.tile([P, P], DT.float16)
            nc.sync.dma_start(iota16_sb[:], iota16_d.ap())
            mu_sb = cp.tile([P, N_BASIS], f32)
            nc.sync.dma_start(mu_sb[:], mu_d.ap())
            mcol_sb = cp.tile([72, 36 * K], r_f)
            if F16_F:
                mcol_f32 = cp.tile([72, 36 * K], f32)
                nc.sync.dma_start(mcol_f32[:], mcol_d.ap())
                nc.vector.tensor_copy(mcol_sb[:], mcol_f32[:])
            else:
                nc.sync.dma_start(mcol_sb[:], mcol_d.ap())
            wcg_sb = cp.tile([K, 3 * K], r_cg)
            if F16_CG:
                wcg_f32 = cp.tile([K, 3 * K], f32)
                nc.sync.dma_start(wcg_f32[:], wcg_d.ap())
                nc.vector.tensor_copy(wcg_sb[:], wcg_f32[:])
            else:
                nc.sync.dma_start(wcg_sb[:], wcg_d.ap())
            eexp_sb = cp.tile([N_TYPES, K0_TOT], DT.float16)
            eexp_f32 = cp.tile([N_TYPES, K0_TOT], f32)
            nc.sync.dma_start(eexp_f32[:], eexp_d.ap())
            nc.vector.tensor_copy(eexp_sb[:], eexp_f32[:])
            whead_sb = [cp.tile([K, K0_TOT], r_hd, name=f"whead{i}", tag=f"whead{i}") for i in range(3)]
            for i in range(3):
                if F16_HEAD:
                    wtmp = cp.tile([K, K0_TOT], f32, tag=f"wheadf{i}")
                    nc.sync.dma_start(wtmp[:], whead_d.ap()[i])
                    nc.vector.tensor_copy(whead_sb[i][:], wtmp[:])
                else:
                    nc.sync.dma_start(whead_sb[i][:], whead_d.ap()[i])
            bhead_sb = cp.tile([K, 3], f32)
            nc.sync.dma_start(bhead_sb[:], bhead_d.ap())
            wout_sb = cp.tile([K, 3], r_hd)
            if F16_HEAD:
                wout_f32 = cp.tile([K, 3], f32)
                nc.sync.dma_start(wout_f32[:], wout_d.ap())
                nc.vector.tensor_copy(wout_sb[:], wout_f32[:])
            else:
                nc.sync.dma_start(wout_sb[:], wout_d.ap())
            bout_sb = cp.tile([1, 1], f32)
            nc.sync.dma_start(bout_sb[:], bout_d.ap())
            specr_sb = cp.tile([N_TYPES, NS], f32)
            nc.sync.dma_start(specr_sb[:], specr_d.ap())
            svals_sb = cp.tile([N_TYPES, 1], f32)
            nc.sync.dma_start(svals_sb[:], svals_d.ap())

            def bias_tile(val, tag):
                bt = cp.tile([P, 1], f32, tag=tag)
                nc.vector.memset(bt[:], val)
                return bt

            b_eps = bias_tile(1e-12, "b_eps")
            b_half_pi = bias_tile(float(np.pi / 2), "b_hpi")
            b_zero = bias_tile(0.0, "b_zero")

            # ---- G accumulator in SBUF ----
            g_sb = gp.tile([72, NBLK * P], r_f)

            # ================= pair stage =================
            # Phase A/B: whole-T geometry + activations batched by table set
            # (Ln/Exp together, then Sin) to avoid ACT table thrash.
            with tc.tile_pool(name="geom", bufs=1) as gm:
                pnb = gm.tile([P, T, 3], f32)
                nc.sync.dma_start(pnb[:], posnb_d.ap())
                pct = gm.tile([P, T, 3], f32)
                nc.sync.dma_start(pct[:], posct_d.ap())
                colT = gm.tile([P, T], DT.float16)
                nc.sync.dma_start(colT[:], colf_d.ap())

                rv = gm.tile([P, T, 3], f32)
                nc.vector.tensor_tensor(out=rv[:], in0=pnb[:], in1=pct[:],
                                        op=ALU.subtract)
                rr = gm.tile([P, T], f32)
                nc.vector.tensor_tensor(out=rr[:], in0=rv[:, :, 0],
                                        in1=rv[:, :, 0], op=ALU.mult)
                tmp2 = gm.tile([P, T], f32)
                nc.vector.tensor_tensor(out=tmp2[:], in0=rv[:, :, 1],
                                        in1=rv[:, :, 1], op=ALU.mult)
                nc.vector.tensor_tensor(out=rr[:], in0=rr[:], in1=tmp2[:],
                                        op=ALU.add)
                nc.vector.tensor_tensor(out=tmp2[:], in0=rv[:, :, 2],
                                        in1=rv[:, :, 2], op=ALU.mult)
                nc.vector.tensor_tensor(out=rr[:], in0=rr[:], in1=tmp2[:],
                                        op=ALU.add)
                lnrr = gm.tile([P, T], f32)
                nc.scalar.activation(lnrr[:], rr[:], AF.Ln,
                                     bias=b_eps[:], scale=1.0)
                dd = gm.tile([P, T], f32)
                nc.scalar.activation(dd[:], lnrr[:], AF.Exp,
                                     bias=b_zero[:], scale=0.5)
                invd = gm.tile([P, T], f32)
                nc.scalar.activation(invd[:], lnrr[:], AF.Exp,
                                     bias=b_zero[:], scale=-0.5)
                uv = gm.tile([P, T, 3], f32)
                nc.vector.tensor_tensor(
                    out=uv[:], in0=rv[:],
                    in1=invd[:].unsqueeze(2).to_broadcast([P, T, 3]),
                    op=ALU.mult)

                sh = gm.tile([P, 8, T], f32)
                ux, uy, uz = uv[:, :, 0], uv[:, :, 1], uv[:, :, 2]
                nc.vector.tensor_copy(sh[:, 0, :], uy)
                nc.vector.tensor_copy(sh[:, 1, :], uz)
                nc.vector.tensor_copy(sh[:, 2, :], ux)
                nc.vector.scalar_tensor_tensor(out=sh[:, 3, :], in0=ux,
                                               scalar=SQ3, in1=uy,
                                               op0=ALU.mult, op1=ALU.mult)
                nc.vector.scalar_tensor_tensor(out=sh[:, 4, :], in0=uy,
                                               scalar=SQ3, in1=uz,
                                               op0=ALU.mult, op1=ALU.mult)
                zz3 = gm.tile([P, T], f32)
                nc.vector.scalar_tensor_tensor(out=zz3[:], in0=uz,
                                               scalar=3.0, in1=uz,
                                               op0=ALU.mult, op1=ALU.mult)
                nc.vector.tensor_scalar(out=sh[:, 5, :], in0=zz3[:],
                                        scalar1=0.5, scalar2=-0.5,
                                        op0=ALU.mult, op1=ALU.add)
                nc.vector.scalar_tensor_tensor(out=sh[:, 6, :], in0=ux,
                                               scalar=SQ3, in1=uz,
                                               op0=ALU.mult, op1=ALU.mult)
                xx = gm.tile([P, T], f32)
                nc.vector.scalar_tensor_tensor(out=xx[:], in0=ux,
                                               scalar=0.5 * SQ3, in1=ux,
                                               op0=ALU.mult, op1=ALU.mult)
                yy = gm.tile([P, T], f32)
                nc.vector.scalar_tensor_tensor(out=yy[:], in0=uy,
                                               scalar=0.5 * SQ3, in1=uy,
                                               op0=ALU.mult, op1=ALU.mult)
                nc.vector.tensor_tensor(out=sh[:, 7, :], in0=xx[:],
                                        in1=yy[:], op=ALU.subtract)

                ev = gm.tile([P, N_BASIS, T], f32)
                nc.vector.tensor_tensor(
                    out=ev[:],
                    in0=dd[:].unsqueeze(1).to_broadcast([P, N_BASIS, T]),
                    in1=mu_sb[:].unsqueeze(2).to_broadcast([P, N_BASIS, T]),
                    op=ALU.subtract)
                e2 = gm.tile([P, N_BASIS, T], f32)
                nc.vector.tensor_tensor(out=e2[:], in0=ev[:], in1=ev[:],
                                        op=ALU.mult)
                gauss = gm.tile([P, N_BASIS, T], f32)
                nc.scalar.activation(gauss[:], e2[:], AF.Exp,
                                     bias=b_zero[:],
                                     scale=-1.0 / (SIGMA * SIGMA))
                tcv = gm.tile([P, T], f32)
                nc.vector.tensor_scalar(out=tcv[:], in0=dd[:],
                                        scalar1=CUTOFF - CUTOFF_WIDTH,
                                        scalar2=1.0 / CUTOFF_WIDTH,
                                        op0=ALU.subtract, op1=ALU.mult)
                nc.vector.tensor_scalar(out=tcv[:], in0=tcv[:],
                                        scalar1=0.0, scalar2=1.0,
                                        op0=ALU.max, op1=ALU.min)
                cosv = gm.tile([P, T], f32)
                nc.scalar.activation(cosv[:], tcv[:], AF.Sin,
                                     bias=b_half_pi[:],
                                     scale=-float(np.pi))
                fc = gm.tile([P, T], f32)
                nc.vector.tensor_scalar(out=fc[:], in0=cosv[:],
                                        scalar1=0.5, scalar2=0.5,
                                        op0=ALU.mult, op1=ALU.add)
                rb = gm.tile([P, N_BASIS, T], f32)
                nc.vector.tensor_tensor(
                    out=rb[:], in0=gauss[:],
                    in1=fc[:].unsqueeze(1).to_broadcast([P, N_BASIS, T]),
                    op=ALU.mult)

                # Phase C: chunked V/S build + scatter matmuls
                with tc.tile_pool(name="pair", bufs=2) as wp:
                    for ch in range(NCH):
                        t0 = ch * TC
                        vt = wp.tile([P, TC, 72], r_sc)
                        nc.vector.tensor_copy(
                            vt[:, :, 0:8],
                            rb[:, :, t0:t0 + TC].rearrange("p b t -> p t b"))
                        nc.vector.tensor_tensor(
                            out=vt[:, :, 8:72].rearrange(
                                "p t (lm b) -> p t lm b", lm=8, b=8),
                            in0=sh[:, :, t0:t0 + TC]
                                .rearrange("p lm t -> p t lm")
                                .unsqueeze(3).to_broadcast([P, TC, 8, 8]),
                            in1=rb[:, :, t0:t0 + TC]
                                .rearrange("p b t -> p t b")
                                .unsqueeze(2).to_broadcast([P, TC, 8, 8]),
                            op=ALU.mult)
                        st = wp.tile([P, TC, P], r_sc)
                        nc.vector.tensor_tensor(
                            out=st[:],
                            in0=colT[:, t0:t0 + TC].unsqueeze(2)
                                .to_broadcast([P, TC, P]),
                            in1=iota16_sb[:].unsqueeze(1)
                                .to_broadcast([P, TC, P]),
                            op=ALU.is_equal)
                        for bl in range(BPC):
                            b = ch * BPC + bl
                            psg = pp.tile([72, P], f32, space="PSUM",
                                          tag="psG")
                            for j in range(TPB):
                                tt = bl * TPB + j
                                nc.tensor.matmul(out=psg[:],
                                                 lhsT=vt[:, tt, :],
                                                 rhs=st[:, tt, :],
                                                 start=(j == 0),
                                                 stop=(j == TPB - 1))
                            nc.scalar.copy(g_sb[:, b * P:(b + 1) * P], psg[:])

            # ================= atom stage =================
            with tc.tile_pool(name="atom", bufs=1) as ap:
                # f matmuls: fT[k, lm, slot]
                g4 = g_sb[:].rearrange("p (blk a s) -> p blk a s",
                                       a=A_BLK, s=N_TYPES)
                csizes = [(i, min(16, NBLK - i)) for i in range(0, NBLK, 16)]
                csizes2 = [(i, min(32, NBLK - i)) for i in range(0, NBLK, 32)]
                ft = ap.tile([K, 9, NS], r_cg)
                for lm in range(9):
                    for (cb0, cbn) in csizes:
                        psf = pp.tile([K, 512], f32, space="PSUM", tag="ps512")
                        n = cbn * A_BLK
                        for s in range(N_TYPES):
                            nc.tensor.matmul(
                                out=psf[:, 0:n],
                                lhsT=mcol_sb[:, (lm * 4 + s) * K:(lm * 4 + s + 1) * K],
                                rhs=g4[:, cb0:cb0 + cbn, :, s],
                                start=(s == 0), stop=(s == N_TYPES - 1))
                        nc.scalar.copy(ft[:, lm, cb0 * A_BLK:cb0 * A_BLK + n],
                                       psf[:, 0:n])

                # CG stage: tl[k, l, slot]
                tl = ap.tile([K, 3, NS], f32)
                tmp = ap.tile([K, 512], f32)
                for l in range(3):
                    lms = [i for i in range(9) if L_OF_LM[i] == l]
                    for (cb0, cbn) in csizes:
                        n = cbn * A_BLK
                        sl = slice(cb0 * A_BLK, cb0 * A_BLK + n)
                        for mi, lm in enumerate(lms):
                            psc = pp.tile([K, 512], f32, space="PSUM",
                                          tag="ps512")
                            nc.tensor.matmul(
                                out=psc[:, 0:n],
                                lhsT=wcg_sb[:, l * K:(l + 1) * K],
                                rhs=ft[:, lm, sl],
                                start=True, stop=True)
                            if mi == 0:
                                nc.vector.tensor_tensor(
                                    out=tl[:, l, sl], in0=psc[:, 0:n],
                                    in1=ft[:, lm, sl], op=ALU.mult)
                            else:
                                nc.vector.tensor_tensor(
                                    out=tmp[:, 0:n], in0=psc[:, 0:n],
                                    in1=ft[:, lm, sl], op=ALU.mult)
                                nc.vector.tensor_tensor(
                                    out=tl[:, l, sl], in0=tl[:, l, sl],
                                    in1=tmp[:, 0:n], op=ALU.add)
                        if l == 0:
                            nc.vector.tensor_tensor(
                                out=tl[:, 0, sl], in0=tl[:, 0, sl],
                                in1=ft[:, 0, sl], op=ALU.add)

                # species one-hot of centers and embedding factor
                oct_sb = ap.tile([N_TYPES, NS], DT.float16)
                nc.vector.tensor_tensor(
                    out=oct_sb[:], in0=specr_sb[:],
                    in1=svals_sb[:].to_broadcast([N_TYPES, NS]),
                    op=ALU.is_equal)
                x0e = ap.tile([K, 3, NS], r_hd)
                for l in range(3):
                    for (cb0, cbn) in csizes:
                        n = cbn * A_BLK
                        sl = slice(cb0 * A_BLK, cb0 * A_BLK + n)
                        pse = pp.tile([K, 512], f32, space="PSUM", tag="ps512")
                        nc.tensor.matmul(out=pse[:, 0:n],
                                         lhsT=eexp_sb[:, l * K:(l + 1) * K],
                                         rhs=oct_sb[:, sl],
                                         start=True, stop=True)
                        nc.vector.tensor_tensor(out=x0e[:, l, sl],
                                                in0=pse[:, 0:n],
                                                in1=tl[:, l, sl], op=ALU.mult)

                # head: hT[j, slot] = silu(sum_R Whead[R, j] x0e[R, slot] + b)
                ht = ap.tile([K, 3, NS], r_hd)
                for jc in range(3):
                    for (cb0, cbn) in csizes:
                        n = cbn * A_BLK
                        sl = slice(cb0 * A_BLK, cb0 * A_BLK + n)
                        psh = pp.tile([K, 512], f32, space="PSUM", tag="ps512")
                        for rc in range(3):
                            nc.tensor.matmul(
                                out=psh[:, 0:n],
                                lhsT=whead_sb[rc][:, jc * K:(jc + 1) * K],
                                rhs=x0e[:, rc, sl],
                                start=(rc == 0), stop=(rc == 2))
                        nc.scalar.activation(ht[:, jc, sl], psh[:, 0:n],
                                             AF.Silu,
                                             bias=bhead_sb[:, jc:jc + 1],
                                             scale=1.0)

                # out row
                outsb = ap.tile([1, NS], f32)
                ocs = [(i, min(16, NBLK - i)) for i in range(0, NBLK, 16)]
                for (cb0, cbn) in ocs:
                    n = cbn * A_BLK
                    sl = slice(cb0 * A_BLK, cb0 * A_BLK + n)
                    pso = pp.tile([1, 512], f32, space="PSUM", tag="psO", bufs=1)
                    for rc in range(3):
                        nc.tensor.matmul(out=pso[:, 0:n],
                                         lhsT=wout_sb[:, rc:rc + 1],
                                         rhs=ht[:, rc, sl],
                                         start=(rc == 0), stop=(rc == 2))
                    nc.scalar.activation(outsb[:, sl], pso[:, 0:n],
                                         AF.Identity,
                                         bias=bout_sb[:], scale=1.0)
                nc.sync.dma_start(out_d.ap(), outsb[:])

    nc.compile()
    return nc, T


def _prep_inputs(inputs, TPB):
    """Host-side sharding: sort pairs by center, bucket into per-core,
    per-block tile slots, and materialize per-pair endpoint positions."""
    T = NBLK * TPB
    pos = np.ascontiguousarray(np.asarray(inputs["positions"], np.float32))
    spec = np.asarray(inputs["species"]).astype(np.int64)
    pairs = np.asarray(inputs["pairs"]).astype(np.int64)
    ctr, nbr = pairs[:, 0], pairs[:, 1]
    order = np.argsort(ctr, kind="stable")
    ctr = ctr[order]
    nbr = nbr[order]
    spec_nb = spec[nbr].astype(np.float32)

    core = ctr // NLOC
    loc = ctr - core * NLOC
    blk = loc // A_BLK
    arel = loc - blk * A_BLK

    # rank within (core, block)
    key = core * NBLK + blk
    # pairs sorted by ctr -> key is non-decreasing
    counts = np.bincount(key, minlength=NCORES * NBLK)
    starts = np.concatenate([[0], np.cumsum(counts)[:-1]])
    rank = np.arange(len(ctr)) - starts[key]

    slot = blk * (TPB * P) + rank          # slot within core's pair arrays
    tt = slot // P
    qq = slot - tt * P

    in_maps = []
    # constant tables (shared across cores)
    iota_np = np.broadcast_to(np.arange(P, dtype=np.float32), (P, P)).copy()
    mu_np = np.broadcast_to(
        np.linspace(0.0, CUTOFF, N_BASIS, dtype=np.float32), (P, N_BASIS)).copy()

    emb = np.asarray(inputs["embeddings"], np.float32)
    h0t = np.repeat(emb, N_MAX, axis=1)                    # [4, 128]
    W_rad = np.asarray(inputs["W_rad"], np.float32)
    mcol = np.zeros((72, 36 * K), np.float32)
    for lm in range(9):
        l = L_OF_LM[lm]
        for s in range(N_TYPES):
            blkc = (lm * 4 + s) * K
            for b in range(N_BASIS):
                mcol[lm * 8 + b, blkc:blkc + K] = \
                    MP_SCALING * W_rad[l, b, :] * h0t[s, :]
    wcg = np.concatenate([
        np.asarray(inputs["W_cg0"], np.float32),
        np.asarray(inputs["W_cg1"], np.float32) * np.float32(-1.0 / SQ3),
        np.asarray(inputs["W_cg2"], np.float32) * np.float32(1.0 / SQ3),
    ], axis=1)                                             # [128, 384]
    eexp = np.repeat(emb, K0_TOT // N_CHANNELS, axis=1)    # [4, 384]
    W_head = np.asarray(inputs["W_head"], np.float32)      # [384, 384]
    whead = np.stack([W_head[i * K:(i + 1) * K, :] for i in range(3)])
    b_head = np.asarray(inputs["b_head"], np.float32)
    bhead = b_head.reshape(3, K).T.copy()                  # [128, 3]
    W_out = np.asarray(inputs["W_out"], np.float32)        # [384, 1]
    wout = W_out[:, 0].reshape(3, K).T.copy()              # [128, 3]
    bout = np.asarray(inputs["b_out"], np.float32).reshape(1, 1)

    for c in range(NCORES):
        m = core == c
        posnb = np.zeros((P, T, 3), np.float32)
        posct = np.zeros((P, T, 3), np.float32)
        colf = np.full((P, T), -1.0, np.float16)
        posnb[qq[m], tt[m]] = pos[nbr[m]]
        posct[qq[m], tt[m]] = pos[ctr[m]]
        colf[qq[m], tt[m]] = (arel[m] * N_TYPES + spec_nb[m]).astype(np.float16)
        slots = np.arange(NS)
        atom = c * NLOC + np.minimum(slots, NLOC - 1)
        specr = np.broadcast_to(spec[atom].astype(np.float32), (N_TYPES, NS)).copy()
        in_maps.append(dict(
            posnb=posnb, posct=posct, colf=colf, specr=specr,
            iota=iota_np, iota16=iota_np.astype(np.float16),
            mu=mu_np, mcol=mcol, wcg=wcg, eexp=eexp,
            whead=whead, bhead=bhead, wout=wout, bout=bout,
            svals=np.arange(N_TYPES, dtype=np.float32).reshape(N_TYPES, 1),
        ))
    return in_maps


def _required_tpb(inputs):
    pairs = np.asarray(inputs["pairs"]).astype(np.int64)
    ctr = pairs[:, 0]
    key = (ctr // NLOC) * NBLK + (ctr % NLOC) // A_BLK
    counts = np.bincount(key, minlength=NCORES * NBLK)
    return max(5, int(math.ceil(counts.max() / P)))


def _install_ntff_hook():
    """Provide the antenv.axon_hooks registry this image lacks, backed by
    direct ctypes calls into libaxon_pjrt.so (same mechanism trn_boot uses)."""
    import types
    if "antenv.axon_hooks" in sys.modules:
        return
    try:
        import antenv
        from trn_agent_boot.trn_boot import _ntff_profile_via_ctypes
        hook = _ntff_profile_via_ctypes("/opt/axon/libaxon_pjrt.so")
        mod = types.ModuleType("antenv.axon_hooks")
        _h = {"hook": hook}
        mod.get_axon_ntff_profile_hook = lambda: _h["hook"]
        mod.set_axon_ntff_profile_hook = lambda h: _h.__setitem__("hook", h)
        sys.modules["antenv.axon_hooks"] = mod
        antenv.axon_hooks = mod
        bass_utils.upload_artifacts = lambda d: f"file://{d}"
    except Exception as e:
        print("ntff hook install failed:", repr(e))


def run_cores(inputs, trace=False):
    if trace:
        _install_ntff_hook()
    TPB = _required_tpb(inputs)
    if TPB not in _BUILD_CACHE:
        _BUILD_CACHE[TPB] = _build(TPB)
    nc, T = _BUILD_CACHE[TPB]
    in_maps = _prep_inputs(inputs, TPB)
    res = bass_utils.run_bass_kernel_spmd(
        nc, in_maps, core_ids=list(range(NCORES)), trace=trace)
    outs = [res.results[c]["out"][0, :NLOC] for c in range(NCORES)]
    full = np.concatenate(outs).reshape(N_ATOMS, 1).astype(np.float32)
    return full, res


def kernel(**inputs):
    full, _ = run_cores(inputs, trace=False)
    return full
